# revision 12
# baseline (speedup 1.0000x reference)
"""Trainium2 Bass kernel for nn_MultiHeadCrossAttention_67963562492589.

Reference computation (B=16, S=1024, H=4, QD=128, KD=VD=256):
    tq = (query @ Wq + bq).view(B, H, 1024, 256)   # torch .view semantics!
    tk = (key   @ Wk + bk).view(B, H, 1024, 256)
    tv = (value @ Wv + bv).view(B, H, 1024, 256)
    scores   = tq @ tk^T          (no 1/sqrt(d) scaling)
    attn     = softmax(scores, -1)                  # [B,H,1024,1024] OUTPUT
    attended = attn @ tv  -> .view(B, 1024, 1024)
    out      = ((attended @ Wo + bo).mean(1)) @ W1 + b1   # [B,128] OUTPUT

Key algebraic facts used:
  * The .view head-split means head h covers flat rows h*1024..h*1024+1023 of
    the [4096, 256] projected matrix; flat row r = original position s = r//4
    and feature-quarter j = r%4.  Head h attends over s in [h*256, (h+1)*256).
  * mean-before-matmul: `out` depends on attn only through per-(b,h) column
    sums of attn grouped by (row mod 4):
        R[j, k] = sum_{q = j mod 4} attn[q, k]     (tiny mask matmul, with
                                                    1/rowsum folded into the
                                                    mask weights)
        Z[j, :] = sum_h sum_k R[j,k] * tv_h[k, :]
        out     = (Z.flatten()/1024) @ (Wo @ W1) + (bo @ W1 + b1)
    so `attended` is never materialized and Wo/W1 fold into one [1024,128]
    constant computed on the host (the 1/1024 is folded into it too).

Sharding: pure data parallel - batch 16 -> 2 per core across 8 cores.
Matmuls run in bf16 (PSUM accumulates fp32; fp32r measured 2x slower on
silicon).  The host pre-transposes q/k/v into the [d, s] layouts the
TensorEngine needs and pre-casts inputs/weights to bf16 (pure layout/dtype
prep - every FLOP of the reference computation runs on device).  Softmax is
exp-without-max-subtraction (|scores| stays small for any sane input scale),
with the normalization applied by DVE/GpSimd and folded into the R mask.
"""

import numpy as np

B, S, H = 16, 1024, 4
QD, KD, VD = 128, 256, 256
N_CORES = 8
B_LOC = B // N_CORES  # 2 batches per core

_CACHE = {}


def _build_nc():
    import concourse.mybir as mybir
    import concourse.tile as tile
    from concourse import bacc
    from contextlib import ExitStack

    f32 = mybir.dt.float32
    bf16 = mybir.dt.bfloat16
    AF = mybir.ActivationFunctionType

    nc = bacc.Bacc("TRN2", target_bir_lowering=False, debug=False,
                   num_devices=N_CORES)

    # ---- DRAM parameters -------------------------------------------------
    # qT[b]  : [128(qd), 1024(s)]          = query[b].T          (bf16)
    # kT[b]  : [2(c), 128(dl), 1024(s)],  kT[b,c,p,s] = key[b,s,c*128+p]
    # vT[b]  : same layout as kT
    qT_d = nc.dram_tensor("qT", [B_LOC, 128, S], bf16, kind="ExternalInput").ap()
    kT_d = nc.dram_tensor("kT", [B_LOC, 2, 128, S], bf16,
                          kind="ExternalInput").ap()
    vT_d = nc.dram_tensor("vT", [B_LOC, 2, 128, S], bf16,
                          kind="ExternalInput").ap()
    wq_d = nc.dram_tensor("Wq", [QD, H * KD], bf16, kind="ExternalInput").ap()
    wk_d = nc.dram_tensor("Wk", [KD, H * KD], bf16, kind="ExternalInput").ap()
    wv_d = nc.dram_tensor("Wv", [VD, H * VD], bf16, kind="ExternalInput").ap()
    wc_d = nc.dram_tensor("Wc", [1024, 128], bf16, kind="ExternalInput").ap()
    bq_d = nc.dram_tensor("bq", [1024], f32, kind="ExternalInput").ap()
    bk_d = nc.dram_tensor("bk", [1024], f32, kind="ExternalInput").ap()
    bv_d = nc.dram_tensor("bv", [1024], bf16, kind="ExternalInput").ap()
    bc_d = nc.dram_tensor("bc", [128], bf16, kind="ExternalInput").ap()
    eye_d = nc.dram_tensor("eye", [128, 128], f32, kind="ExternalInput").ap()
    m01_d = nc.dram_tensor("mask01", [128, 4], bf16, kind="ExternalInput").ap()

    attn_d = nc.dram_tensor("attn", [B_LOC, H, S, S], f32,
                            kind="ExternalOutput").ap()
    out_d = nc.dram_tensor("out", [B_LOC, 128], f32,
                           kind="ExternalOutput").ap()

    with tile.TileContext(nc) as tc, ExitStack() as ctx:
        const = ctx.enter_context(tc.tile_pool(name="const", bufs=1))
        trp = ctx.enter_context(tc.tile_pool(name="trp", bufs=2))
        proj = ctx.enter_context(tc.tile_pool(name="proj", bufs=1))
        small = ctx.enter_context(tc.tile_pool(name="small", bufs=2))
        stat = ctx.enter_context(tc.tile_pool(name="stat", bufs=6))
        expp = ctx.enter_context(tc.tile_pool(name="expp", bufs=5))
        attp = ctx.enter_context(tc.tile_pool(name="attp", bufs=5))

        ps_sc = ctx.enter_context(
            tc.tile_pool(name="ps_sc", bufs=2, space="PSUM"))
        ps_r = ctx.enter_context(
            tc.tile_pool(name="ps_r", bufs=1, space="PSUM"))
        ps_m = ctx.enter_context(
            tc.tile_pool(name="ps_m", bufs=2, space="PSUM"))

        # ---- constants / weights into SBUF (all pre-cast on host) --------
        eye_s = const.tile([128, 128], f32, tag="eye")
        nc.sync.dma_start(eye_s[:], eye_d[:])
        wq_s = const.tile([128, 1024], bf16, tag="wq")
        nc.sync.dma_start(wq_s[:], wq_d[:])
        wk_s = const.tile([128, 2048], bf16, tag="wk")
        nc.sync.dma_start(wk_s[:].rearrange("p (c f) -> p c f", c=2),
                          wk_d.rearrange("(c p) f -> p c f", p=128))
        wv_s = const.tile([128, 2048], bf16, tag="wv")
        nc.sync.dma_start(wv_s[:].rearrange("p (c f) -> p c f", c=2),
                          wv_d.rearrange("(c p) f -> p c f", p=128))
        wc_s = const.tile([128, 1024], bf16, tag="wc")
        nc.sync.dma_start(wc_s[:].rearrange("p (t f) -> p t f", t=8),
                          wc_d.rearrange("(t p) f -> p t f", p=128))
        bq_c = const.tile([128, 8], f32, tag="bqc")
        nc.sync.dma_start(bq_c[:], bq_d.rearrange("(t p) -> p t", p=128))
        bk_c = const.tile([128, 8], f32, tag="bkc")
        nc.sync.dma_start(bk_c[:], bk_d.rearrange("(t p) -> p t", p=128))
        bv_row = const.tile([1, 1024], bf16, tag="bvr")
        nc.sync.dma_start(bv_row[0:1, :], bv_d[:])
        bc_row = const.tile([1, 128], bf16, tag="bcr")
        nc.sync.dma_start(bc_row[0:1, :], bc_d[:])
        m01_s = const.tile([128, 4], bf16, tag="m01")
        nc.sync.dma_start(m01_s[:], m01_d[:])
        ones_s = const.tile([1, 128], bf16, tag="ones")
        nc.vector.memset(ones_s[:], 1.0)

        for b in range(B_LOC):
            # ---- load pre-transposed inputs ------------------------------
            qT = trp.tile([128, 1024], bf16, tag="qT")
            nc.sync.dma_start(qT[:], qT_d[b])
            kT = trp.tile([128, 2, 1024], bf16, tag="kT")
            nc.sync.dma_start(kT[:], kT_d[b].rearrange("c p s -> p c s"))
            vT = trp.tile([128, 2, 1024], bf16, tag="vT")
            nc.sync.dma_start(vT[:], vT_d[b].rearrange("c p s -> p c s"))

            # ---- projections ---------------------------------------------
            # head-major layouts (q' = qq*4 + j, f = j*256 + c*128 + p,
            # s = h*256 + qq):
            #   tqT[p, c, h, q'] = tq_hT[d = c*128+p, q']   (same for tkT)
            #   tv[p, st, f]     = tv[s = st*128+p, f]      (natural form)
            tqT = proj.tile([128, 2, 4, 1024], bf16, tag="tqT")
            tkT = proj.tile([128, 2, 4, 1024], bf16, tag="tkT")
            tv = proj.tile([128, 8, 1024], bf16, tag="tv")

            def evac_bias(dst, ps, bias_ap, idx):
                # PSUM -> SBUF copy + per-partition bias add + bf16 round,
                # alternating engines to balance DVE/ACT load.
                if idx % 2 == 0:
                    nc.vector.tensor_scalar_add(dst, ps, bias_ap)
                else:
                    nc.scalar.activation(dst, ps, AF.Identity, bias=bias_ap)

            # evac scatters the [f-tile, s-chunk] psum tile into head-major
            # layout: m -> (j = m//2, c = m%2); an s-chunk covers two heads
            # (hh outer, qq inner matches psum element order).
            tqT_w = tqT[:].rearrange("p c hh (qq j) -> p c hh qq j", j=4)
            tkT_w = tkT[:].rearrange("p c hh (qq j) -> p c hh qq j", j=4)
            idx = 0
            for m in range(8):
                j, c = m // 2, m % 2
                for sc in range(2):
                    ps = ps_m.tile([128, 512], f32, tag="misc")
                    nc.tensor.matmul(
                        ps[:], wq_s[:, m * 128:(m + 1) * 128],
                        qT[:, sc * 512:(sc + 1) * 512],
                        start=True, stop=True)
                    evac_bias(tqT_w[:, c, sc * 2:(sc + 1) * 2, :, j], ps[:],
                              bq_c[:, m:m + 1], idx)
                    idx += 1
            for m in range(8):
                j, c2 = m // 2, m % 2
                for sc in range(2):
                    ps = ps_m.tile([128, 512], f32, tag="misc")
                    for c in range(2):
                        nc.tensor.matmul(
                            ps[:],
                            wk_s[:, c * 1024 + m * 128:
                                 c * 1024 + (m + 1) * 128],
                            kT[:, c, sc * 512:(sc + 1) * 512],
                            start=(c == 0), stop=(c == 1))
                    evac_bias(tkT_w[:, c2, sc * 2:(sc + 1) * 2, :, j], ps[:],
                              bk_c[:, m:m + 1], idx)
                    idx += 1
            for st in range(8):
                for fc in range(2):
                    ps = ps_m.tile([128, 512], f32, tag="misc")
                    for c in range(2):
                        nc.tensor.matmul(
                            ps[:],
                            vT[:, c, st * 128:(st + 1) * 128],
                            wv_s[:, c * 1024 + fc * 512:
                                 c * 1024 + fc * 512 + 512],
                            start=(c == 0), stop=False)
                    # + bv broadcast along partitions via rank-1 accumulate
                    nc.tensor.matmul(
                        ps[:], ones_s[0:1, :],
                        bv_row[0:1, fc * 512:(fc + 1) * 512],
                        start=False, stop=True)
                    nc.any.tensor_copy(tv[:, st, fc * 512:(fc + 1) * 512],
                                       ps[:])

            # ---- attention per head --------------------------------------
            Z_sb = small.tile([4, 256], f32, tag="Z")
            for h in range(H):
                R_ps = ps_r.tile([4, 1024], f32, tag="R")
                for qt in range(8):
                    sc_ps = ps_sc.tile([128, 1024], f32, tag="sc")
                    for c in range(2):
                        lhs = tqT[:, c, h, qt * 128:(qt + 1) * 128]
                        for nch in range(2):
                            rhs = tkT[:, c, h, nch * 512:(nch + 1) * 512]
                            nc.tensor.matmul(
                                sc_ps[:, nch * 512:(nch + 1) * 512],
                                lhs, rhs,
                                start=(c == 0), stop=(c == 1))
                    # softmax (no max subtraction: |scores| is small)
                    exp_t = expp.tile([128, 1024], bf16, tag="exp")
                    rowsum = stat.tile([128, 1], f32, tag="rs")
                    nc.scalar.activation(exp_t[:], sc_ps[:], AF.Exp,
                                         accum_out=rowsum[:])
                    recip = stat.tile([128, 1], f32, tag="rc")
                    nc.vector.reciprocal(recip[:], rowsum[:])
                    attn_t = attp.tile([128, 1024], f32, tag="attn")
                    if qt % 2 == 0:
                        nc.vector.tensor_scalar_mul(attn_t[:], exp_t[:],
                                                    recip[:])
                    else:
                        nc.gpsimd.tensor_scalar_mul(attn_t[:], exp_t[:],
                                                    recip[:])
                    nc.sync.dma_start(
                        attn_d[b, h, qt * 128:(qt + 1) * 128, :], attn_t[:])
                    # R[j,k] += sum_{q'=j mod 4} attn[q',k], with the 1/rowsum
                    # folded into the mask weights so R reads bf16 exp.
                    msc = stat.tile([128, 4], bf16, tag="msc")
                    nc.vector.tensor_scalar_mul(msc[:], m01_s[:], recip[:])
                    for nch in range(2):
                        nc.tensor.matmul(
                            R_ps[0:4, nch * 512:(nch + 1) * 512],
                            msc[:],
                            exp_t[:, nch * 512:(nch + 1) * 512],
                            start=(qt == 0), stop=(qt == 7))

                # Z[j,:] += sum_{k} R[j,k] tv_h[k,:] for this head
                R_sb = small.tile([4, 1024], f32, tag="Rsb")
                nc.any.tensor_copy(R_sb[:], R_ps[:])
                R_r = R_sb[:].rearrange("p (kk i) -> p i kk", i=4)
                RT = small.tile([128, 8, 4], bf16, tag="RT")
                for i in range(4):
                    for kc in range(2):
                        ps = ps_m.tile([128, 128], f32, tag="misc")
                        nc.tensor.transpose(
                            ps[0:128, 0:4],
                            R_r[0:4, i, kc * 128:(kc + 1) * 128],
                            eye_s[0:4, 0:4])
                        nc.any.tensor_copy(RT[:, i * 2 + kc, :],
                                           ps[0:128, 0:4])
                Z_ps = ps_m.tile([128, 512], f32, tag="misc")
                n = 0
                for i in range(4):
                    for kc in range(2):
                        nc.tensor.matmul(
                            Z_ps[0:4, 0:256],
                            RT[:, i * 2 + kc, :],
                            tv[:, 2 * h + kc, i * 256:(i + 1) * 256],
                            start=(n == 0), stop=(n == 7))
                        n += 1
                if h == 0:
                    nc.vector.tensor_copy(Z_sb[:], Z_ps[0:4, 0:256])
                else:
                    nc.vector.tensor_add(Z_sb[:], Z_sb[:], Z_ps[0:4, 0:256])

            # ---- out head: out[b] = (Z.flat/1024) @ (Wo@W1) + bc ---------
            # (the 1/1024 is folded into Wc on the host)
            # attmean[256*j + hf*128 + p] = Z[j, hf*128 + p] = zT[hf][p, j]
            zT = small.tile([128, 2, 4], bf16, tag="zT")
            for hf in range(2):
                ps = ps_m.tile([128, 128], f32, tag="misc")
                nc.tensor.transpose(
                    ps[0:128, 0:4],
                    Z_sb[0:4, hf * 128:(hf + 1) * 128],
                    eye_s[0:4, 0:4])
                nc.any.tensor_copy(zT[:, hf, :], ps[0:128, 0:4])
            out_ps = ps_m.tile([128, 512], f32, tag="misc")
            for t in range(8):
                j, hf = t // 2, t % 2
                nc.tensor.matmul(
                    out_ps[0:1, 0:128],
                    zT[:, hf, j:j + 1],
                    wc_s[:, t * 128:(t + 1) * 128],
                    start=(t == 0), stop=False)
            nc.tensor.matmul(out_ps[0:1, 0:128], ones_s[0:1, 0:1],
                             bc_row[0:1, :], start=False, stop=True)
            out_sb = small.tile([1, 128], f32, tag="outsb")
            nc.any.tensor_copy(out_sb[:], out_ps[0:1, 0:128])
            nc.sync.dma_start(out_d[b:b + 1, :], out_sb[:])

    nc.compile()
    return nc


def _get_nc():
    if "nc" not in _CACHE:
        _CACHE["nc"] = _build_nc()
    return _CACHE["nc"]


def _make_in_maps(inputs):
    import ml_dtypes

    bf16 = ml_dtypes.bfloat16
    q = np.asarray(inputs["query"], dtype=np.float32)
    k = np.asarray(inputs["key"], dtype=np.float32)
    v = np.asarray(inputs["value"], dtype=np.float32)
    Wq = np.asarray(inputs["Wq"], dtype=np.float32)
    Wk = np.asarray(inputs["Wk"], dtype=np.float32)
    Wv = np.asarray(inputs["Wv"], dtype=np.float32)
    bq = np.asarray(inputs["bq"], dtype=np.float32)
    bk = np.asarray(inputs["bk"], dtype=np.float32)
    bv = np.asarray(inputs["bv"], dtype=np.float32)
    Wo = np.asarray(inputs["Wo"], dtype=np.float64)
    bo = np.asarray(inputs["bo"], dtype=np.float64)
    W1 = np.asarray(inputs["W1"], dtype=np.float64)
    b1 = np.asarray(inputs["b1"], dtype=np.float64)

    # layout/dtype prep only (all FLOPs of the reference run on device,
    # except the constant folding Wc = Wo @ W1 which is weight-only)
    qT = np.ascontiguousarray(q.transpose(0, 2, 1)).astype(bf16)
    kT = np.ascontiguousarray(
        k.reshape(B, S, 2, 128).transpose(0, 2, 3, 1)).astype(bf16)
    vT = np.ascontiguousarray(
        v.reshape(B, S, 2, 128).transpose(0, 2, 3, 1)).astype(bf16)

    Wc = ((Wo @ W1) / 1024.0).astype(bf16)
    bc = (bo @ W1 + b1).astype(bf16)
    eye = np.eye(128, dtype=np.float32)
    mask01 = np.zeros((128, 4), dtype=bf16)
    mask01[np.arange(128), np.arange(128) % 4] = 1.0

    shared = {"Wq": Wq.astype(bf16), "Wk": Wk.astype(bf16),
              "Wv": Wv.astype(bf16), "Wc": Wc, "bq": bq, "bk": bk,
              "bv": bv.astype(bf16), "bc": bc, "eye": eye, "mask01": mask01}
    in_maps = []
    for c in range(N_CORES):
        sl = slice(c * B_LOC, (c + 1) * B_LOC)
        in_maps.append({"qT": qT[sl], "kT": kT[sl], "vT": vT[sl], **shared})
    return in_maps


def _run(inputs, trace=False, **kw):
    from concourse.bass_utils import run_bass_kernel_spmd

    nc = _get_nc()
    in_maps = _make_in_maps(inputs)
    res = run_bass_kernel_spmd(nc, in_maps, core_ids=list(range(N_CORES)),
                               trace=trace, **kw)
    attn = np.empty((B, H, S, S), dtype=np.float32)
    out = np.empty((B, 128), dtype=np.float32)
    for c in range(N_CORES):
        sl = slice(c * B_LOC, (c + 1) * B_LOC)
        attn[sl] = res.results[c]["attn"]
        out[sl] = res.results[c]["out"]
    return (out, attn), res


def kernel(**inputs):
    (out, attn), _ = _run(inputs)
    return out, attn


# revision 13
# speedup vs baseline: 2.3888x; 2.3888x over previous
"""Trainium2 Bass kernel for nn_MultiHeadCrossAttention_67963562492589.

Reference computation (B=16, S=1024, H=4, QD=128, KD=VD=256):
    tq = (query @ Wq + bq).view(B, H, 1024, 256)   # torch .view semantics!
    tk = (key   @ Wk + bk).view(B, H, 1024, 256)
    tv = (value @ Wv + bv).view(B, H, 1024, 256)
    scores   = tq @ tk^T          (no 1/sqrt(d) scaling)
    attn     = softmax(scores, -1)                  # [B,H,1024,1024] OUTPUT
    attended = attn @ tv  -> .view(B, 1024, 1024)
    out      = ((attended @ Wo + bo).mean(1)) @ W1 + b1   # [B,128] OUTPUT

Key algebraic facts used:
  * The .view head-split means head h covers flat rows h*1024..h*1024+1023 of
    the [4096, 256] projected matrix; flat row r = original position s = r//4
    and feature-quarter j = r%4.  Head h attends over s in [h*256, (h+1)*256).
  * mean-before-matmul: `out` depends on attn only through per-(b,h) column
    sums of attn grouped by (row mod 4):
        R[j, k] = sum_{q = j mod 4} attn[q, k]     (tiny mask matmul, with
                                                    1/rowsum folded into the
                                                    mask weights)
        Z[j, :] = sum_h sum_k R[j,k] * tv_h[k, :]
        out     = (Z.flatten()/1024) @ (Wo @ W1) + (bo @ W1 + b1)
    so `attended` is never materialized and Wo/W1 fold into one [1024,128]
    constant computed on the host (the 1/1024 is folded into it too).

Sharding: pure data parallel - batch 16 -> 2 per core across 8 cores.
Matmuls run in bf16 (PSUM accumulates fp32; fp32r measured 2x slower on
silicon).  The host pre-transposes q/k/v into the [d, s] layouts the
TensorEngine needs and pre-casts inputs/weights to bf16 (pure layout/dtype
prep - every FLOP of the reference computation runs on device).  Softmax is
exp-without-max-subtraction (|scores| stays small for any sane input scale),
with the normalization applied by DVE/GpSimd and folded into the R mask.
"""

import numpy as np

B, S, H = 16, 1024, 4
QD, KD, VD = 128, 256, 256
N_CORES = 8
B_LOC = B // N_CORES  # 2 batches per core

_CACHE = {}


def _build_nc():
    import concourse.mybir as mybir
    import concourse.tile as tile
    from concourse import bacc
    from contextlib import ExitStack

    f32 = mybir.dt.float32
    bf16 = mybir.dt.bfloat16
    AF = mybir.ActivationFunctionType

    nc = bacc.Bacc("TRN2", target_bir_lowering=False, debug=False,
                   num_devices=N_CORES)

    # ---- DRAM parameters -------------------------------------------------
    # qT[b]  : [128(qd), 1024(s)]          = query[b].T          (bf16)
    # kT[b]  : [2(c), 128(dl), 1024(s)],  kT[b,c,p,s] = key[b,s,c*128+p]
    # vT[b]  : same layout as kT
    qT_d = nc.dram_tensor("qT", [B_LOC, 128, S], bf16, kind="ExternalInput").ap()
    kT_d = nc.dram_tensor("kT", [B_LOC, 2, 128, S], bf16,
                          kind="ExternalInput").ap()
    vT_d = nc.dram_tensor("vT", [B_LOC, 2, 128, S], bf16,
                          kind="ExternalInput").ap()
    wq_d = nc.dram_tensor("Wq", [QD, H * KD], bf16, kind="ExternalInput").ap()
    wk_d = nc.dram_tensor("Wk", [KD, H * KD], bf16, kind="ExternalInput").ap()
    wv_d = nc.dram_tensor("Wv", [VD, H * VD], bf16, kind="ExternalInput").ap()
    wc_d = nc.dram_tensor("Wc", [1024, 128], bf16, kind="ExternalInput").ap()
    bq_d = nc.dram_tensor("bq", [1024], f32, kind="ExternalInput").ap()
    bk_d = nc.dram_tensor("bk", [1024], f32, kind="ExternalInput").ap()
    bv_d = nc.dram_tensor("bv", [1024], bf16, kind="ExternalInput").ap()
    bc_d = nc.dram_tensor("bc", [128], bf16, kind="ExternalInput").ap()
    eye_d = nc.dram_tensor("eye", [128, 128], f32, kind="ExternalInput").ap()
    m01_d = nc.dram_tensor("mask01", [128, 4], bf16, kind="ExternalInput").ap()

    attn_d = nc.dram_tensor("attn", [B_LOC, H, S, S], f32,
                            kind="ExternalOutput").ap()
    out_d = nc.dram_tensor("out", [B_LOC, 128], f32,
                           kind="ExternalOutput").ap()

    with tile.TileContext(nc) as tc, ExitStack() as ctx:
        const = ctx.enter_context(tc.tile_pool(name="const", bufs=1))
        trp = ctx.enter_context(tc.tile_pool(name="trp", bufs=2))
        proj = ctx.enter_context(tc.tile_pool(name="proj", bufs=1))
        small = ctx.enter_context(tc.tile_pool(name="small", bufs=2))
        stat = ctx.enter_context(tc.tile_pool(name="stat", bufs=6))
        expp = ctx.enter_context(tc.tile_pool(name="expp", bufs=5))
        attp = ctx.enter_context(tc.tile_pool(name="attp", bufs=5))

        ps_sc = ctx.enter_context(
            tc.tile_pool(name="ps_sc", bufs=2, space="PSUM"))
        ps_r = ctx.enter_context(
            tc.tile_pool(name="ps_r", bufs=1, space="PSUM"))
        ps_m = ctx.enter_context(
            tc.tile_pool(name="ps_m", bufs=2, space="PSUM"))

        # ---- constants / weights into SBUF (all pre-cast on host) --------
        eye_s = const.tile([128, 128], f32, tag="eye")
        nc.sync.dma_start(eye_s[:], eye_d[:])
        wq_s = const.tile([128, 1024], bf16, tag="wq")
        nc.sync.dma_start(wq_s[:], wq_d[:])
        wk_s = const.tile([128, 2048], bf16, tag="wk")
        nc.sync.dma_start(wk_s[:].rearrange("p (c f) -> p c f", c=2),
                          wk_d.rearrange("(c p) f -> p c f", p=128))
        wv_s = const.tile([128, 2048], bf16, tag="wv")
        nc.sync.dma_start(wv_s[:].rearrange("p (c f) -> p c f", c=2),
                          wv_d.rearrange("(c p) f -> p c f", p=128))
        wc_s = const.tile([128, 1024], bf16, tag="wc")
        nc.sync.dma_start(wc_s[:].rearrange("p (t f) -> p t f", t=8),
                          wc_d.rearrange("(t p) f -> p t f", p=128))
        bq_c = const.tile([128, 8], f32, tag="bqc")
        nc.sync.dma_start(bq_c[:], bq_d.rearrange("(t p) -> p t", p=128))
        bk_c = const.tile([128, 8], f32, tag="bkc")
        nc.sync.dma_start(bk_c[:], bk_d.rearrange("(t p) -> p t", p=128))
        bv_row = const.tile([1, 1024], bf16, tag="bvr")
        nc.sync.dma_start(bv_row[0:1, :], bv_d[:])
        bc_row = const.tile([1, 128], bf16, tag="bcr")
        nc.sync.dma_start(bc_row[0:1, :], bc_d[:])
        m01_s = const.tile([128, 4], bf16, tag="m01")
        nc.sync.dma_start(m01_s[:], m01_d[:])
        ones_s = const.tile([1, 128], bf16, tag="ones")
        nc.vector.memset(ones_s[:], 1.0)

        for b in range(B_LOC):
            # ---- load pre-transposed inputs ------------------------------
            qT = trp.tile([128, 1024], bf16, tag="qT")
            nc.sync.dma_start(qT[:], qT_d[b])
            kT = trp.tile([128, 2, 1024], bf16, tag="kT")
            nc.sync.dma_start(kT[:], kT_d[b].rearrange("c p s -> p c s"))
            vT = trp.tile([128, 2, 1024], bf16, tag="vT")
            nc.sync.dma_start(vT[:], vT_d[b].rearrange("c p s -> p c s"))

            # ---- projections ---------------------------------------------
            # head-major layouts (q' = qq*4 + j, f = j*256 + c*128 + p,
            # s = h*256 + qq):
            #   tqT[p, c, h, q'] = tq_hT[d = c*128+p, q']   (same for tkT)
            #   tv[p, st, f]     = tv[s = st*128+p, f]      (natural form)
            tqT = proj.tile([128, 2, 4, 1024], bf16, tag="tqT")
            tkT = proj.tile([128, 2, 4, 1024], bf16, tag="tkT")
            tv = proj.tile([128, 8, 1024], bf16, tag="tv")

            def evac_bias(dst, ps, bias_ap, idx):
                # PSUM -> SBUF copy + per-partition bias add + bf16 round,
                # alternating engines to balance DVE/ACT load.
                if idx % 2 == 0:
                    nc.vector.tensor_scalar_add(dst, ps, bias_ap)
                else:
                    nc.scalar.activation(dst, ps, AF.Identity, bias=bias_ap)

            # evac scatters the [f-tile, s-chunk] psum tile into head-major
            # layout: m -> (j = m//2, c = m%2); an s-chunk covers two heads
            # (hh outer, qq inner matches psum element order).
            tqT_w = tqT[:].rearrange("p c hh (qq j) -> p c hh qq j", j=4)
            tkT_w = tkT[:].rearrange("p c hh (qq j) -> p c hh qq j", j=4)
            idx = 0
            for m in range(8):
                j, c = m // 2, m % 2
                for sc in range(2):
                    ps = ps_m.tile([128, 512], f32, tag="misc")
                    nc.tensor.matmul(
                        ps[:], wq_s[:, m * 128:(m + 1) * 128],
                        qT[:, sc * 512:(sc + 1) * 512],
                        start=True, stop=True)
                    evac_bias(tqT_w[:, c, sc * 2:(sc + 1) * 2, :, j], ps[:],
                              bq_c[:, m:m + 1], idx)
                    idx += 1
            for m in range(8):
                j, c2 = m // 2, m % 2
                for sc in range(2):
                    ps = ps_m.tile([128, 512], f32, tag="misc")
                    for c in range(2):
                        nc.tensor.matmul(
                            ps[:],
                            wk_s[:, c * 1024 + m * 128:
                                 c * 1024 + (m + 1) * 128],
                            kT[:, c, sc * 512:(sc + 1) * 512],
                            start=(c == 0), stop=(c == 1))
                    evac_bias(tkT_w[:, c2, sc * 2:(sc + 1) * 2, :, j], ps[:],
                              bk_c[:, m:m + 1], idx)
                    idx += 1
            for st in range(8):
                for fc in range(2):
                    ps = ps_m.tile([128, 512], f32, tag="misc")
                    for c in range(2):
                        nc.tensor.matmul(
                            ps[:],
                            vT[:, c, st * 128:(st + 1) * 128],
                            wv_s[:, c * 1024 + fc * 512:
                                 c * 1024 + fc * 512 + 512],
                            start=(c == 0), stop=False)
                    # + bv broadcast along partitions via rank-1 accumulate
                    nc.tensor.matmul(
                        ps[:], ones_s[0:1, :],
                        bv_row[0:1, fc * 512:(fc + 1) * 512],
                        start=False, stop=True)
                    nc.any.tensor_copy(tv[:, st, fc * 512:(fc + 1) * 512],
                                       ps[:])

            # ---- attention per head --------------------------------------
            Z_sb = small.tile([4, 256], f32, tag="Z")
            for h in range(H):
                R_ps = ps_r.tile([4, 1024], f32, tag="R")
                for qt in range(8):
                    sc_ps = ps_sc.tile([128, 1024], f32, tag="sc")
                    for c in range(2):
                        lhs = tqT[:, c, h, qt * 128:(qt + 1) * 128]
                        for nch in range(2):
                            rhs = tkT[:, c, h, nch * 512:(nch + 1) * 512]
                            nc.tensor.matmul(
                                sc_ps[:, nch * 512:(nch + 1) * 512],
                                lhs, rhs,
                                start=(c == 0), stop=(c == 1))
                    # softmax (no max subtraction: |scores| is small)
                    exp_t = expp.tile([128, 1024], bf16, tag="exp")
                    rowsum = stat.tile([128, 1], f32, tag="rs")
                    nc.scalar.activation(exp_t[:], sc_ps[:], AF.Exp,
                                         accum_out=rowsum[:])
                    recip = stat.tile([128, 1], f32, tag="rc")
                    nc.vector.reciprocal(recip[:], rowsum[:])
                    # normalize in bf16 (DVE 4x mode); the f32 conversion for
                    # the attn output happens inside the casting SWDGE store.
                    attn_t = attp.tile([128, 1024], bf16, tag="attn")
                    nc.vector.tensor_scalar_mul(attn_t[:], exp_t[:], recip[:])
                    nc.gpsimd.dma_start(
                        attn_d[b, h, qt * 128:(qt + 1) * 128, :], attn_t[:])
                    # R[j,k] += sum_{q'=j mod 4} attn[q',k]
                    for nch in range(2):
                        nc.tensor.matmul(
                            R_ps[0:4, nch * 512:(nch + 1) * 512],
                            m01_s[:],
                            attn_t[:, nch * 512:(nch + 1) * 512],
                            start=(qt == 0), stop=(qt == 7))

                # Z[j,:] += sum_{k} R[j,k] tv_h[k,:] for this head
                R_sb = small.tile([4, 1024], f32, tag="Rsb")
                nc.any.tensor_copy(R_sb[:], R_ps[:])
                R_r = R_sb[:].rearrange("p (kk i) -> p i kk", i=4)
                RT = small.tile([128, 8, 4], bf16, tag="RT")
                for i in range(4):
                    for kc in range(2):
                        ps = ps_m.tile([128, 128], f32, tag="misc")
                        nc.tensor.transpose(
                            ps[0:128, 0:4],
                            R_r[0:4, i, kc * 128:(kc + 1) * 128],
                            eye_s[0:4, 0:4])
                        nc.any.tensor_copy(RT[:, i * 2 + kc, :],
                                           ps[0:128, 0:4])
                Z_ps = ps_m.tile([128, 512], f32, tag="misc")
                n = 0
                for i in range(4):
                    for kc in range(2):
                        nc.tensor.matmul(
                            Z_ps[0:4, 0:256],
                            RT[:, i * 2 + kc, :],
                            tv[:, 2 * h + kc, i * 256:(i + 1) * 256],
                            start=(n == 0), stop=(n == 7))
                        n += 1
                if h == 0:
                    nc.vector.tensor_copy(Z_sb[:], Z_ps[0:4, 0:256])
                else:
                    nc.vector.tensor_add(Z_sb[:], Z_sb[:], Z_ps[0:4, 0:256])

            # ---- out head: out[b] = (Z.flat/1024) @ (Wo@W1) + bc ---------
            # (the 1/1024 is folded into Wc on the host)
            # attmean[256*j + hf*128 + p] = Z[j, hf*128 + p] = zT[hf][p, j]
            zT = small.tile([128, 2, 4], bf16, tag="zT")
            for hf in range(2):
                ps = ps_m.tile([128, 128], f32, tag="misc")
                nc.tensor.transpose(
                    ps[0:128, 0:4],
                    Z_sb[0:4, hf * 128:(hf + 1) * 128],
                    eye_s[0:4, 0:4])
                nc.any.tensor_copy(zT[:, hf, :], ps[0:128, 0:4])
            out_ps = ps_m.tile([128, 512], f32, tag="misc")
            for t in range(8):
                j, hf = t // 2, t % 2
                nc.tensor.matmul(
                    out_ps[0:1, 0:128],
                    zT[:, hf, j:j + 1],
                    wc_s[:, t * 128:(t + 1) * 128],
                    start=(t == 0), stop=False)
            nc.tensor.matmul(out_ps[0:1, 0:128], ones_s[0:1, 0:1],
                             bc_row[0:1, :], start=False, stop=True)
            out_sb = small.tile([1, 128], f32, tag="outsb")
            nc.any.tensor_copy(out_sb[:], out_ps[0:1, 0:128])
            nc.sync.dma_start(out_d[b:b + 1, :], out_sb[:])

    nc.compile()
    return nc


def _get_nc():
    if "nc" not in _CACHE:
        _CACHE["nc"] = _build_nc()
    return _CACHE["nc"]


def _make_in_maps(inputs):
    import ml_dtypes

    bf16 = ml_dtypes.bfloat16
    q = np.asarray(inputs["query"], dtype=np.float32)
    k = np.asarray(inputs["key"], dtype=np.float32)
    v = np.asarray(inputs["value"], dtype=np.float32)
    Wq = np.asarray(inputs["Wq"], dtype=np.float32)
    Wk = np.asarray(inputs["Wk"], dtype=np.float32)
    Wv = np.asarray(inputs["Wv"], dtype=np.float32)
    bq = np.asarray(inputs["bq"], dtype=np.float32)
    bk = np.asarray(inputs["bk"], dtype=np.float32)
    bv = np.asarray(inputs["bv"], dtype=np.float32)
    Wo = np.asarray(inputs["Wo"], dtype=np.float64)
    bo = np.asarray(inputs["bo"], dtype=np.float64)
    W1 = np.asarray(inputs["W1"], dtype=np.float64)
    b1 = np.asarray(inputs["b1"], dtype=np.float64)

    # layout/dtype prep only (all FLOPs of the reference run on device,
    # except the constant folding Wc = Wo @ W1 which is weight-only)
    qT = np.ascontiguousarray(q.transpose(0, 2, 1)).astype(bf16)
    kT = np.ascontiguousarray(
        k.reshape(B, S, 2, 128).transpose(0, 2, 3, 1)).astype(bf16)
    vT = np.ascontiguousarray(
        v.reshape(B, S, 2, 128).transpose(0, 2, 3, 1)).astype(bf16)

    Wc = ((Wo @ W1) / 1024.0).astype(bf16)
    bc = (bo @ W1 + b1).astype(bf16)
    eye = np.eye(128, dtype=np.float32)
    mask01 = np.zeros((128, 4), dtype=bf16)
    mask01[np.arange(128), np.arange(128) % 4] = 1.0

    shared = {"Wq": Wq.astype(bf16), "Wk": Wk.astype(bf16),
              "Wv": Wv.astype(bf16), "Wc": Wc, "bq": bq, "bk": bk,
              "bv": bv.astype(bf16), "bc": bc, "eye": eye, "mask01": mask01}
    in_maps = []
    for c in range(N_CORES):
        sl = slice(c * B_LOC, (c + 1) * B_LOC)
        in_maps.append({"qT": qT[sl], "kT": kT[sl], "vT": vT[sl], **shared})
    return in_maps


def _run(inputs, trace=False, **kw):
    from concourse.bass_utils import run_bass_kernel_spmd

    nc = _get_nc()
    in_maps = _make_in_maps(inputs)
    res = run_bass_kernel_spmd(nc, in_maps, core_ids=list(range(N_CORES)),
                               trace=trace, **kw)
    attn = np.empty((B, H, S, S), dtype=np.float32)
    out = np.empty((B, 128), dtype=np.float32)
    for c in range(N_CORES):
        sl = slice(c * B_LOC, (c + 1) * B_LOC)
        attn[sl] = res.results[c]["attn"]
        out[sl] = res.results[c]["out"]
    return (out, attn), res


def kernel(**inputs):
    (out, attn), _ = _run(inputs)
    return out, attn


# revision 17
# speedup vs baseline: 2.5325x; 1.0602x over previous
"""Trainium2 Bass kernel for nn_MultiHeadCrossAttention_67963562492589.

Reference computation (B=16, S=1024, H=4, QD=128, KD=VD=256):
    tq = (query @ Wq + bq).view(B, H, 1024, 256)   # torch .view semantics!
    tk = (key   @ Wk + bk).view(B, H, 1024, 256)
    tv = (value @ Wv + bv).view(B, H, 1024, 256)
    scores   = tq @ tk^T          (no 1/sqrt(d) scaling)
    attn     = softmax(scores, -1)                  # [B,H,1024,1024] OUTPUT
    attended = attn @ tv  -> .view(B, 1024, 1024)
    out      = ((attended @ Wo + bo).mean(1)) @ W1 + b1   # [B,128] OUTPUT

Key algebraic facts used:
  * The .view head-split means head h covers flat rows h*1024..h*1024+1023 of
    the [4096, 256] projected matrix; flat row r = original position s = r//4
    and feature-quarter j = r%4.  Head h attends over s in [h*256, (h+1)*256).
  * mean-before-matmul: `out` depends on attn only through per-(b,h) column
    sums of attn grouped by (row mod 4):
        R[j, k] = sum_{q = j mod 4} attn[q, k]     (tiny mask matmul, with
                                                    1/rowsum folded into the
                                                    mask weights)
        Z[j, :] = sum_h sum_k R[j,k] * tv_h[k, :]
        out     = (Z.flatten()/1024) @ (Wo @ W1) + (bo @ W1 + b1)
    so `attended` is never materialized and Wo/W1 fold into one [1024,128]
    constant computed on the host (the 1/1024 is folded into it too).

Sharding: pure data parallel - batch 16 -> 2 per core across 8 cores.
Matmuls run in bf16 (PSUM accumulates fp32; fp32r measured 2x slower on
silicon).  The host pre-transposes q/k/v into the [d, s] layouts the
TensorEngine needs and pre-casts inputs/weights to bf16 (pure layout/dtype
prep - every FLOP of the reference computation runs on device).  Softmax is
exp-without-max-subtraction (|scores| stays small for any sane input scale),
with the normalization applied by DVE/GpSimd and folded into the R mask.
"""

import numpy as np

B, S, H = 16, 1024, 4
QD, KD, VD = 128, 256, 256
N_CORES = 8
B_LOC = B // N_CORES  # 2 batches per core

_CACHE = {}


def _build_nc():
    import concourse.mybir as mybir
    import concourse.tile as tile
    from concourse import bacc
    from contextlib import ExitStack

    f32 = mybir.dt.float32
    bf16 = mybir.dt.bfloat16
    AF = mybir.ActivationFunctionType

    nc = bacc.Bacc("TRN2", target_bir_lowering=False, debug=False,
                   num_devices=N_CORES)

    # ---- DRAM parameters -------------------------------------------------
    # qT[b]  : [128(qd), 1024(s)]          = query[b].T          (bf16)
    # kT[b]  : [2(c), 128(dl), 1024(s)],  kT[b,c,p,s] = key[b,s,c*128+p]
    # vT[b]  : same layout as kT
    qT_d = nc.dram_tensor("qT", [B_LOC, 128, S], bf16, kind="ExternalInput").ap()
    kT_d = nc.dram_tensor("kT", [B_LOC, 2, 128, S], bf16,
                          kind="ExternalInput").ap()
    vT_d = nc.dram_tensor("vT", [B_LOC, 2, 128, S], bf16,
                          kind="ExternalInput").ap()
    wq_d = nc.dram_tensor("Wq", [QD, H * KD], bf16, kind="ExternalInput").ap()
    wk_d = nc.dram_tensor("Wk", [KD, H * KD], bf16, kind="ExternalInput").ap()
    wv_d = nc.dram_tensor("Wv", [VD, H * VD], bf16, kind="ExternalInput").ap()
    wc_d = nc.dram_tensor("Wc", [1024, 128], bf16, kind="ExternalInput").ap()
    bq_d = nc.dram_tensor("bq", [1024], f32, kind="ExternalInput").ap()
    bk_d = nc.dram_tensor("bk", [1024], f32, kind="ExternalInput").ap()
    bv_d = nc.dram_tensor("bv", [1024], bf16, kind="ExternalInput").ap()
    bc_d = nc.dram_tensor("bc", [128], bf16, kind="ExternalInput").ap()
    eye_d = nc.dram_tensor("eye", [128, 128], f32, kind="ExternalInput").ap()
    m01_d = nc.dram_tensor("mask01", [128, 4], bf16, kind="ExternalInput").ap()

    attn_d = nc.dram_tensor("attn", [B_LOC, H, S, S], f32,
                            kind="ExternalOutput").ap()
    out_d = nc.dram_tensor("out", [B_LOC, 128], f32,
                           kind="ExternalOutput").ap()

    with tile.TileContext(nc) as tc, ExitStack() as ctx:
        const = ctx.enter_context(tc.tile_pool(name="const", bufs=1))
        trp = ctx.enter_context(tc.tile_pool(name="trp", bufs=2))
        proj = ctx.enter_context(tc.tile_pool(name="proj", bufs=1))
        tvp = ctx.enter_context(tc.tile_pool(name="tvp", bufs=2))
        small = ctx.enter_context(tc.tile_pool(name="small", bufs=2))
        stat = ctx.enter_context(tc.tile_pool(name="stat", bufs=6))
        expp = ctx.enter_context(tc.tile_pool(name="expp", bufs=5))
        attp = ctx.enter_context(tc.tile_pool(name="attp", bufs=5))

        ps_sc = ctx.enter_context(
            tc.tile_pool(name="ps_sc", bufs=2, space="PSUM"))
        ps_r = ctx.enter_context(
            tc.tile_pool(name="ps_r", bufs=1, space="PSUM"))
        ps_m = ctx.enter_context(
            tc.tile_pool(name="ps_m", bufs=2, space="PSUM"))

        # ---- constants / weights into SBUF (all pre-cast on host) --------
        eye_s = const.tile([128, 128], f32, tag="eye")
        nc.sync.dma_start(eye_s[:], eye_d[:])
        wq_s = const.tile([128, 1024], bf16, tag="wq")
        nc.sync.dma_start(wq_s[:], wq_d[:])
        wk_s = const.tile([128, 2048], bf16, tag="wk")
        nc.sync.dma_start(wk_s[:].rearrange("p (c f) -> p c f", c=2),
                          wk_d.rearrange("(c p) f -> p c f", p=128))
        wv_s = const.tile([128, 2048], bf16, tag="wv")
        nc.sync.dma_start(wv_s[:].rearrange("p (c f) -> p c f", c=2),
                          wv_d.rearrange("(c p) f -> p c f", p=128))
        wc_s = const.tile([128, 1024], bf16, tag="wc")
        nc.sync.dma_start(wc_s[:].rearrange("p (t f) -> p t f", t=8),
                          wc_d.rearrange("(t p) f -> p t f", p=128))
        bq_c = const.tile([128, 8], f32, tag="bqc")
        nc.sync.dma_start(bq_c[:], bq_d.rearrange("(t p) -> p t", p=128))
        bk_c = const.tile([128, 8], f32, tag="bkc")
        nc.sync.dma_start(bk_c[:], bk_d.rearrange("(t p) -> p t", p=128))
        bv_row = const.tile([1, 1024], bf16, tag="bvr")
        nc.sync.dma_start(bv_row[0:1, :], bv_d[:])
        bc_row = const.tile([1, 128], bf16, tag="bcr")
        nc.sync.dma_start(bc_row[0:1, :], bc_d[:])
        m01_s = const.tile([128, 4], bf16, tag="m01")
        nc.sync.dma_start(m01_s[:], m01_d[:])
        ones_s = const.tile([1, 128], bf16, tag="ones")
        nc.vector.memset(ones_s[:], 1.0)

        pending = []  # deferred tail-stage emission closures (see below)
        for b in range(B_LOC):
            # ---- load pre-transposed inputs ------------------------------
            qT = trp.tile([128, 1024], bf16, tag="qT")
            nc.sync.dma_start(qT[:], qT_d[b])
            kT = trp.tile([128, 2, 1024], bf16, tag="kT")
            nc.sync.dma_start(kT[:], kT_d[b].rearrange("c p s -> p c s"))
            vT = trp.tile([128, 2, 1024], bf16, tag="vT")
            nc.sync.dma_start(vT[:], vT_d[b].rearrange("c p s -> p c s"))

            # ---- projections ---------------------------------------------
            # head-major layouts (q' = qq*4 + j, f = j*256 + c*128 + p,
            # s = h*256 + qq):
            #   tqT[p, c, h, q'] = tq_hT[d = c*128+p, q']   (same for tkT)
            #   tv[p, st, f]     = tv[s = st*128+p, f]      (natural form)
            tqT = proj.tile([128, 2, 4, 1024], bf16, tag="tqT")
            tkT = proj.tile([128, 2, 4, 1024], bf16, tag="tkT")
            tv = tvp.tile([128, 8, 1024], bf16, tag="tv")

            def evac_bias(dst, ps, bias_ap, idx):
                # PSUM -> SBUF copy + per-partition bias add + bf16 round,
                # alternating engines to balance DVE/ACT load.
                if idx % 2 == 0:
                    nc.vector.tensor_scalar_add(dst, ps, bias_ap)
                else:
                    nc.scalar.activation(dst, ps, AF.Identity, bias=bias_ap)

            # evac scatters the [f-tile, s-chunk] psum tile into head-major
            # layout: m -> (j = m//2, c = m%2); an s-chunk covers two heads
            # (hh outer, qq inner matches psum element order).
            tqT_w = tqT[:].rearrange("p c hh (qq j) -> p c hh qq j", j=4)
            tkT_w = tkT[:].rearrange("p c hh (qq j) -> p c hh qq j", j=4)
            idx = 0
            for m in range(8):
                j, c = m // 2, m % 2
                for sc in range(2):
                    ps = ps_m.tile([128, 512], f32, tag="misc")
                    nc.tensor.matmul(
                        ps[:], wq_s[:, m * 128:(m + 1) * 128],
                        qT[:, sc * 512:(sc + 1) * 512],
                        start=True, stop=True)
                    evac_bias(tqT_w[:, c, sc * 2:(sc + 1) * 2, :, j], ps[:],
                              bq_c[:, m:m + 1], idx)
                    idx += 1
            for m in range(8):
                j, c2 = m // 2, m % 2
                for sc in range(2):
                    ps = ps_m.tile([128, 512], f32, tag="misc")
                    for c in range(2):
                        nc.tensor.matmul(
                            ps[:],
                            wk_s[:, c * 1024 + m * 128:
                                 c * 1024 + (m + 1) * 128],
                            kT[:, c, sc * 512:(sc + 1) * 512],
                            start=(c == 0), stop=(c == 1))
                    evac_bias(tkT_w[:, c2, sc * 2:(sc + 1) * 2, :, j], ps[:],
                              bk_c[:, m:m + 1], idx)
                    idx += 1
            for st in range(8):
                for fc in range(2):
                    ps = ps_m.tile([128, 512], f32, tag="misc")
                    for c in range(2):
                        nc.tensor.matmul(
                            ps[:],
                            vT[:, c, st * 128:(st + 1) * 128],
                            wv_s[:, c * 1024 + fc * 512:
                                 c * 1024 + fc * 512 + 512],
                            start=(c == 0), stop=False)
                    # + bv broadcast along partitions via rank-1 accumulate
                    nc.tensor.matmul(
                        ps[:], ones_s[0:1, :],
                        bv_row[0:1, fc * 512:(fc + 1) * 512],
                        start=False, stop=True)
                    nc.any.tensor_copy(tv[:, st, fc * 512:(fc + 1) * 512],
                                       ps[:])

            # ---- attention per head --------------------------------------
            # The R->RT->Z chain is a latency-bound PE<->DVE ping-pong, so it
            # is emitted one head LATE (into the next head's scores stretch):
            # PE reaches it with every dependency long satisfied and never
            # stalls.  `pending` carries the deferred emission closures.
            Z_sb = small.tile([4, 256], f32, tag="Z")

            def z_stage(h, R_sb, tv, Z_sb):
                def emit():
                    R_r = R_sb[:].rearrange("p (kk i) -> p i kk", i=4)
                    RT = small.tile([128, 8, 4], bf16, tag="RT")
                    for i in range(4):
                        for kc in range(2):
                            ps = ps_m.tile([128, 128], f32, tag="misc")
                            nc.tensor.transpose(
                                ps[0:128, 0:4],
                                R_r[0:4, i, kc * 128:(kc + 1) * 128],
                                eye_s[0:4, 0:4])
                            nc.any.tensor_copy(RT[:, i * 2 + kc, :],
                                               ps[0:128, 0:4])
                    Z_ps = ps_m.tile([128, 512], f32, tag="misc")
                    n = 0
                    for i in range(4):
                        for kc in range(2):
                            nc.tensor.matmul(
                                Z_ps[0:4, 0:256],
                                RT[:, i * 2 + kc, :],
                                tv[:, 2 * h + kc, i * 256:(i + 1) * 256],
                                start=(n == 0), stop=(n == 7))
                            n += 1
                    if h == 0:
                        nc.vector.tensor_copy(Z_sb[:], Z_ps[0:4, 0:256])
                    else:
                        nc.vector.tensor_add(Z_sb[:], Z_sb[:],
                                             Z_ps[0:4, 0:256])
                return emit

            def out_stage(b, Z_sb):
                def emit():
                    # out[b] = (Z.flat/1024) @ (Wo@W1) + bc  (1/1024 in Wc);
                    # attmean[256*j + hf*128 + p] = Z[j, hf*128+p] = zT[hf][p,j]
                    zT = small.tile([128, 2, 4], bf16, tag="zT")
                    for hf in range(2):
                        ps = ps_m.tile([128, 128], f32, tag="misc")
                        nc.tensor.transpose(
                            ps[0:128, 0:4],
                            Z_sb[0:4, hf * 128:(hf + 1) * 128],
                            eye_s[0:4, 0:4])
                        nc.any.tensor_copy(zT[:, hf, :], ps[0:128, 0:4])
                    out_ps = ps_m.tile([128, 512], f32, tag="misc")
                    for t in range(8):
                        j, hf = t // 2, t % 2
                        nc.tensor.matmul(
                            out_ps[0:1, 0:128],
                            zT[:, hf, j:j + 1],
                            wc_s[:, t * 128:(t + 1) * 128],
                            start=(t == 0), stop=False)
                    nc.tensor.matmul(out_ps[0:1, 0:128], ones_s[0:1, 0:1],
                                     bc_row[0:1, :], start=False, stop=True)
                    out_sb = small.tile([1, 128], f32, tag="outsb")
                    nc.any.tensor_copy(out_sb[:], out_ps[0:1, 0:128])
                    nc.sync.dma_start(out_d[b:b + 1, :], out_sb[:])
                return emit

            for h in range(H):
                R_ps = ps_r.tile([4, 1024], f32, tag="R")
                for qt in range(8):
                    sc_ps = ps_sc.tile([128, 1024], f32, tag="sc")
                    for c in range(2):
                        lhs = tqT[:, c, h, qt * 128:(qt + 1) * 128]
                        for nch in range(2):
                            rhs = tkT[:, c, h, nch * 512:(nch + 1) * 512]
                            nc.tensor.matmul(
                                sc_ps[:, nch * 512:(nch + 1) * 512],
                                lhs, rhs,
                                start=(c == 0), stop=(c == 1))
                    # softmax (no max subtraction: |scores| is small)
                    exp_t = expp.tile([128, 1024], bf16, tag="exp")
                    rowsum = stat.tile([128, 1], f32, tag="rs")
                    nc.scalar.activation(exp_t[:], sc_ps[:], AF.Exp,
                                         accum_out=rowsum[:])
                    recip = stat.tile([128, 1], f32, tag="rc")
                    nc.vector.reciprocal(recip[:], rowsum[:])
                    # normalize in bf16 (DVE 4x mode); the f32 conversion for
                    # the attn output happens inside the casting SWDGE store.
                    attn_t = attp.tile([128, 1024], bf16, tag="attn")
                    nc.vector.tensor_scalar_mul(attn_t[:], exp_t[:], recip[:])
                    nc.gpsimd.dma_start(
                        attn_d[b, h, qt * 128:(qt + 1) * 128, :], attn_t[:])
                    # R[j,k] += sum_{q'=j mod 4} attn[q',k]
                    for nch in range(2):
                        nc.tensor.matmul(
                            R_ps[0:4, nch * 512:(nch + 1) * 512],
                            m01_s[:],
                            attn_t[:, nch * 512:(nch + 1) * 512],
                            start=(qt == 0), stop=(qt == 7))

                # free R_ps promptly; the RT/Z consumption is deferred
                R_sb = small.tile([4, 1024], f32, tag="Rsb")
                nc.any.tensor_copy(R_sb[:], R_ps[:])
                if pending:
                    pending.pop(0)()
                pending.append(z_stage(h, R_sb, tv, Z_sb))
            pending.append(out_stage(b, Z_sb))

        while pending:
            pending.pop(0)()

    nc.compile()
    return nc


def _get_nc():
    if "nc" not in _CACHE:
        _CACHE["nc"] = _build_nc()
    return _CACHE["nc"]


def _make_in_maps(inputs):
    import ml_dtypes

    bf16 = ml_dtypes.bfloat16
    q = np.asarray(inputs["query"], dtype=np.float32)
    k = np.asarray(inputs["key"], dtype=np.float32)
    v = np.asarray(inputs["value"], dtype=np.float32)
    Wq = np.asarray(inputs["Wq"], dtype=np.float32)
    Wk = np.asarray(inputs["Wk"], dtype=np.float32)
    Wv = np.asarray(inputs["Wv"], dtype=np.float32)
    bq = np.asarray(inputs["bq"], dtype=np.float32)
    bk = np.asarray(inputs["bk"], dtype=np.float32)
    bv = np.asarray(inputs["bv"], dtype=np.float32)
    Wo = np.asarray(inputs["Wo"], dtype=np.float64)
    bo = np.asarray(inputs["bo"], dtype=np.float64)
    W1 = np.asarray(inputs["W1"], dtype=np.float64)
    b1 = np.asarray(inputs["b1"], dtype=np.float64)

    # layout/dtype prep only (all FLOPs of the reference run on device,
    # except the constant folding Wc = Wo @ W1 which is weight-only)
    qT = np.ascontiguousarray(q.transpose(0, 2, 1)).astype(bf16)
    kT = np.ascontiguousarray(
        k.reshape(B, S, 2, 128).transpose(0, 2, 3, 1)).astype(bf16)
    vT = np.ascontiguousarray(
        v.reshape(B, S, 2, 128).transpose(0, 2, 3, 1)).astype(bf16)

    Wc = ((Wo @ W1) / 1024.0).astype(bf16)
    bc = (bo @ W1 + b1).astype(bf16)
    eye = np.eye(128, dtype=np.float32)
    mask01 = np.zeros((128, 4), dtype=bf16)
    mask01[np.arange(128), np.arange(128) % 4] = 1.0

    shared = {"Wq": Wq.astype(bf16), "Wk": Wk.astype(bf16),
              "Wv": Wv.astype(bf16), "Wc": Wc, "bq": bq, "bk": bk,
              "bv": bv.astype(bf16), "bc": bc, "eye": eye, "mask01": mask01}
    in_maps = []
    for c in range(N_CORES):
        sl = slice(c * B_LOC, (c + 1) * B_LOC)
        in_maps.append({"qT": qT[sl], "kT": kT[sl], "vT": vT[sl], **shared})
    return in_maps


def _run(inputs, trace=False, **kw):
    from concourse.bass_utils import run_bass_kernel_spmd

    nc = _get_nc()
    in_maps = _make_in_maps(inputs)
    res = run_bass_kernel_spmd(nc, in_maps, core_ids=list(range(N_CORES)),
                               trace=trace, **kw)
    attn = np.empty((B, H, S, S), dtype=np.float32)
    out = np.empty((B, 128), dtype=np.float32)
    for c in range(N_CORES):
        sl = slice(c * B_LOC, (c + 1) * B_LOC)
        attn[sl] = res.results[c]["attn"]
        out[sl] = res.results[c]["out"]
    return (out, attn), res


def kernel(**inputs):
    (out, attn), _ = _run(inputs)
    return out, attn


# revision 18
# speedup vs baseline: 2.5369x; 1.0017x over previous
"""Trainium2 Bass kernel for nn_MultiHeadCrossAttention_67963562492589.

Reference computation (B=16, S=1024, H=4, QD=128, KD=VD=256):
    tq = (query @ Wq + bq).view(B, H, 1024, 256)   # torch .view semantics!
    tk = (key   @ Wk + bk).view(B, H, 1024, 256)
    tv = (value @ Wv + bv).view(B, H, 1024, 256)
    scores   = tq @ tk^T          (no 1/sqrt(d) scaling)
    attn     = softmax(scores, -1)                  # [B,H,1024,1024] OUTPUT
    attended = attn @ tv  -> .view(B, 1024, 1024)
    out      = ((attended @ Wo + bo).mean(1)) @ W1 + b1   # [B,128] OUTPUT

Key algebraic facts used:
  * The .view head-split means head h covers flat rows h*1024..h*1024+1023 of
    the [4096, 256] projected matrix; flat row r = original position s = r//4
    and feature-quarter j = r%4.  Head h attends over s in [h*256, (h+1)*256).
  * mean-before-matmul: `out` depends on attn only through per-(b,h) column
    sums of attn grouped by (row mod 4):
        R[j, k] = sum_{q = j mod 4} attn[q, k]     (tiny mask matmul, with
                                                    1/rowsum folded into the
                                                    mask weights)
        Z[j, :] = sum_h sum_k R[j,k] * tv_h[k, :]
        out     = (Z.flatten()/1024) @ (Wo @ W1) + (bo @ W1 + b1)
    so `attended` is never materialized and Wo/W1 fold into one [1024,128]
    constant computed on the host (the 1/1024 is folded into it too).

Sharding: pure data parallel - batch 16 -> 2 per core across 8 cores.
Matmuls run in bf16 (PSUM accumulates fp32; fp32r measured 2x slower on
silicon).  The host pre-transposes q/k/v into the [d, s] layouts the
TensorEngine needs and pre-casts inputs/weights to bf16 (pure layout/dtype
prep - every FLOP of the reference computation runs on device).  Softmax is
exp-without-max-subtraction (|scores| stays small for any sane input scale),
with the normalization applied by DVE/GpSimd and folded into the R mask.
"""

import numpy as np

B, S, H = 16, 1024, 4
QD, KD, VD = 128, 256, 256
N_CORES = 8
B_LOC = B // N_CORES  # 2 batches per core

_CACHE = {}


def _build_nc():
    import concourse.mybir as mybir
    import concourse.tile as tile
    from concourse import bacc
    from contextlib import ExitStack

    f32 = mybir.dt.float32
    bf16 = mybir.dt.bfloat16
    AF = mybir.ActivationFunctionType

    nc = bacc.Bacc("TRN2", target_bir_lowering=False, debug=False,
                   num_devices=N_CORES)

    # ---- DRAM parameters -------------------------------------------------
    # qT[b]  : [128(qd), 1024(s)]          = query[b].T          (bf16)
    # kT[b]  : [2(c), 128(dl), 1024(s)],  kT[b,c,p,s] = key[b,s,c*128+p]
    # vT[b]  : same layout as kT
    qT_d = nc.dram_tensor("qT", [B_LOC, 128, S], bf16, kind="ExternalInput").ap()
    kT_d = nc.dram_tensor("kT", [B_LOC, 2, 128, S], bf16,
                          kind="ExternalInput").ap()
    vT_d = nc.dram_tensor("vT", [B_LOC, 2, 128, S], bf16,
                          kind="ExternalInput").ap()
    wq_d = nc.dram_tensor("Wq", [QD, H * KD], bf16, kind="ExternalInput").ap()
    wk_d = nc.dram_tensor("Wk", [KD, H * KD], bf16, kind="ExternalInput").ap()
    wv_d = nc.dram_tensor("Wv", [VD, H * VD], bf16, kind="ExternalInput").ap()
    wc_d = nc.dram_tensor("Wc", [1024, 128], bf16, kind="ExternalInput").ap()
    bq_d = nc.dram_tensor("bq", [1024], f32, kind="ExternalInput").ap()
    bk_d = nc.dram_tensor("bk", [1024], f32, kind="ExternalInput").ap()
    bv_d = nc.dram_tensor("bv", [1024], bf16, kind="ExternalInput").ap()
    bc_d = nc.dram_tensor("bc", [128], bf16, kind="ExternalInput").ap()
    eye_d = nc.dram_tensor("eye", [128, 128], f32, kind="ExternalInput").ap()
    m01_d = nc.dram_tensor("mask01", [128, 4], bf16, kind="ExternalInput").ap()

    attn_d = nc.dram_tensor("attn", [B_LOC, H, S, S], f32,
                            kind="ExternalOutput").ap()
    out_d = nc.dram_tensor("out", [B_LOC, 128], f32,
                           kind="ExternalOutput").ap()

    with tile.TileContext(nc) as tc, ExitStack() as ctx:
        const = ctx.enter_context(tc.tile_pool(name="const", bufs=1))
        trp = ctx.enter_context(tc.tile_pool(name="trp", bufs=2))
        proj = ctx.enter_context(tc.tile_pool(name="proj", bufs=1))
        tvp = ctx.enter_context(tc.tile_pool(name="tvp", bufs=2))
        small = ctx.enter_context(tc.tile_pool(name="small", bufs=2))
        stat = ctx.enter_context(tc.tile_pool(name="stat", bufs=6))
        expp = ctx.enter_context(tc.tile_pool(name="expp", bufs=5))
        attp = ctx.enter_context(tc.tile_pool(name="attp", bufs=5))

        ps_sc = ctx.enter_context(
            tc.tile_pool(name="ps_sc", bufs=2, space="PSUM"))
        ps_r = ctx.enter_context(
            tc.tile_pool(name="ps_r", bufs=1, space="PSUM"))
        ps_m = ctx.enter_context(
            tc.tile_pool(name="ps_m", bufs=2, space="PSUM"))

        # ---- constants / weights into SBUF (all pre-cast on host) --------
        # (emitted after the first batch's input loads below so the first
        # projection matmuls start as early as possible)
        wq_s = const.tile([128, 1024], bf16, tag="wq")
        nc.sync.dma_start(wq_s[:], wq_d[:])
        wk_s = const.tile([128, 2048], bf16, tag="wk")
        nc.sync.dma_start(wk_s[:].rearrange("p (c f) -> p c f", c=2),
                          wk_d.rearrange("(c p) f -> p c f", p=128))
        wv_s = const.tile([128, 2048], bf16, tag="wv")
        nc.sync.dma_start(wv_s[:].rearrange("p (c f) -> p c f", c=2),
                          wv_d.rearrange("(c p) f -> p c f", p=128))
        wc_s = const.tile([128, 1024], bf16, tag="wc")
        nc.sync.dma_start(wc_s[:].rearrange("p (t f) -> p t f", t=8),
                          wc_d.rearrange("(t p) f -> p t f", p=128))
        bq_c = const.tile([128, 8], f32, tag="bqc")
        nc.sync.dma_start(bq_c[:], bq_d.rearrange("(t p) -> p t", p=128))
        bk_c = const.tile([128, 8], f32, tag="bkc")
        nc.sync.dma_start(bk_c[:], bk_d.rearrange("(t p) -> p t", p=128))
        bv_row = const.tile([1, 1024], bf16, tag="bvr")
        nc.sync.dma_start(bv_row[0:1, :], bv_d[:])
        bc_row = const.tile([1, 128], bf16, tag="bcr")
        nc.sync.dma_start(bc_row[0:1, :], bc_d[:])
        m01_s = const.tile([128, 4], bf16, tag="m01")
        nc.sync.dma_start(m01_s[:], m01_d[:])
        eye_s = const.tile([128, 128], f32, tag="eye")
        nc.sync.dma_start(eye_s[:], eye_d[:])
        ones_s = const.tile([1, 128], bf16, tag="ones")
        nc.vector.memset(ones_s[:], 1.0)

        pending = []  # deferred z/out-stage emission closures

        def load_inputs(b):
            qT = trp.tile([128, 1024], bf16, tag="qT")
            nc.sync.dma_start(qT[:], qT_d[b])
            kT = trp.tile([128, 2, 1024], bf16, tag="kT")
            nc.sync.dma_start(kT[:], kT_d[b].rearrange("c p s -> p c s"))
            vT = trp.tile([128, 2, 1024], bf16, tag="vT")
            nc.sync.dma_start(vT[:], vT_d[b].rearrange("c p s -> p c s"))
            return qT, kT, vT

        evac_ctr = [0]

        def evac(dst, ps, bias_ap=None):
            # PSUM -> SBUF copy (+ optional per-partition bias) + bf16 round,
            # alternating engines to balance DVE/ACT load.
            i = evac_ctr[0]
            evac_ctr[0] += 1
            if bias_ap is None:
                if i % 2 == 0:
                    nc.vector.tensor_copy(dst, ps)
                else:
                    nc.scalar.copy(dst, ps)
            else:
                if i % 2 == 0:
                    nc.vector.tensor_scalar_add(dst, ps, bias_ap)
                else:
                    nc.scalar.activation(dst, ps, AF.Identity, bias=bias_ap)

        def proj_closures(qT, kT, vT):
            """Per-batch projections as a list of single-psum-tile emission
            closures so they can be interleaved into the previous batch's
            attention stretch.  Layouts (q' = qq*4 + j, f = j*256 + c*128 + p,
            s = h*256 + qq):
              tqT[p, c, h, q'] = tq_hT[d = c*128+p, q']   (same for tkT)
              tv[p, st, f]     = tv[s = st*128+p, f]      (natural form)
            """
            tqT = proj.tile([128, 2, 4, 1024], bf16, tag="tqT")
            tkT = proj.tile([128, 2, 4, 1024], bf16, tag="tkT")
            tv = tvp.tile([128, 8, 1024], bf16, tag="tv")
            tqT_w = tqT[:].rearrange("p c hh (qq j) -> p c hh qq j", j=4)
            tkT_w = tkT[:].rearrange("p c hh (qq j) -> p c hh qq j", j=4)
            clos = []

            def q_tile(m, sc):
                j, c = m // 2, m % 2
                ps = ps_m.tile([128, 512], f32, tag="misc")
                nc.tensor.matmul(
                    ps[:], wq_s[:, m * 128:(m + 1) * 128],
                    qT[:, sc * 512:(sc + 1) * 512],
                    start=True, stop=True)
                evac(tqT_w[:, c, sc * 2:(sc + 1) * 2, :, j], ps[:],
                     bq_c[:, m:m + 1])

            def k_tile(m, sc):
                j, c2 = m // 2, m % 2
                ps = ps_m.tile([128, 512], f32, tag="misc")
                for c in range(2):
                    nc.tensor.matmul(
                        ps[:],
                        wk_s[:, c * 1024 + m * 128:c * 1024 + (m + 1) * 128],
                        kT[:, c, sc * 512:(sc + 1) * 512],
                        start=(c == 0), stop=(c == 1))
                evac(tkT_w[:, c2, sc * 2:(sc + 1) * 2, :, j], ps[:],
                     bk_c[:, m:m + 1])

            def v_tile(st, fc):
                ps = ps_m.tile([128, 512], f32, tag="misc")
                for c in range(2):
                    nc.tensor.matmul(
                        ps[:],
                        vT[:, c, st * 128:(st + 1) * 128],
                        wv_s[:, c * 1024 + fc * 512:c * 1024 + fc * 512 + 512],
                        start=(c == 0), stop=False)
                # + bv broadcast along partitions via rank-1 accumulate
                nc.tensor.matmul(
                    ps[:], ones_s[0:1, :],
                    bv_row[0:1, fc * 512:(fc + 1) * 512],
                    start=False, stop=True)
                evac(tv[:, st, fc * 512:(fc + 1) * 512], ps[:])

            for m in range(8):
                for sc in range(2):
                    clos.append(lambda m=m, sc=sc: q_tile(m, sc))
            for m in range(8):
                for sc in range(2):
                    clos.append(lambda m=m, sc=sc: k_tile(m, sc))
            for st in range(8):
                for fc in range(2):
                    clos.append(lambda st=st, fc=fc: v_tile(st, fc))
            return tqT, tkT, tv, clos

        def z_stage(h, R_sb, tv, Z_sb):
            # The R->RT->Z chain is a latency-bound PE<->DVE ping-pong, so it
            # is emitted one head LATE (into the next head's scores stretch):
            # PE reaches it with every dependency long satisfied.
            def emit():
                R_r = R_sb[:].rearrange("p (kk i) -> p i kk", i=4)
                RT = small.tile([128, 8, 4], bf16, tag="RT")
                for i in range(4):
                    for kc in range(2):
                        ps = ps_m.tile([128, 128], f32, tag="misc")
                        nc.tensor.transpose(
                            ps[0:128, 0:4],
                            R_r[0:4, i, kc * 128:(kc + 1) * 128],
                            eye_s[0:4, 0:4])
                        nc.vector.tensor_copy(RT[:, i * 2 + kc, :],
                                              ps[0:128, 0:4])
                Z_ps = ps_m.tile([128, 512], f32, tag="misc")
                n = 0
                for i in range(4):
                    for kc in range(2):
                        nc.tensor.matmul(
                            Z_ps[0:4, 0:256],
                            RT[:, i * 2 + kc, :],
                            tv[:, 2 * h + kc, i * 256:(i + 1) * 256],
                            start=(n == 0), stop=(n == 7))
                        n += 1
                if h == 0:
                    nc.vector.tensor_copy(Z_sb[:], Z_ps[0:4, 0:256])
                else:
                    nc.vector.tensor_add(Z_sb[:], Z_sb[:], Z_ps[0:4, 0:256])
            return emit

        def out_stage(b, Z_sb):
            def emit():
                # out[b] = (Z.flat/1024) @ (Wo@W1) + bc  (1/1024 in Wc);
                # attmean[256*j + hf*128 + p] = Z[j, hf*128+p] = zT[hf][p, j]
                zT = small.tile([128, 2, 4], bf16, tag="zT")
                for hf in range(2):
                    ps = ps_m.tile([128, 128], f32, tag="misc")
                    nc.tensor.transpose(
                        ps[0:128, 0:4],
                        Z_sb[0:4, hf * 128:(hf + 1) * 128],
                        eye_s[0:4, 0:4])
                    nc.vector.tensor_copy(zT[:, hf, :], ps[0:128, 0:4])
                out_ps = ps_m.tile([128, 512], f32, tag="misc")
                for t in range(8):
                    j, hf = t // 2, t % 2
                    nc.tensor.matmul(
                        out_ps[0:1, 0:128],
                        zT[:, hf, j:j + 1],
                        wc_s[:, t * 128:(t + 1) * 128],
                        start=(t == 0), stop=False)
                nc.tensor.matmul(out_ps[0:1, 0:128], ones_s[0:1, 0:1],
                                 bc_row[0:1, :], start=False, stop=True)
                out_sb = small.tile([1, 128], f32, tag="outsb")
                nc.vector.tensor_copy(out_sb[:], out_ps[0:1, 0:128])
                nc.sync.dma_start(out_d[b:b + 1, :], out_sb[:])
            return emit

        # batch 0: inputs + projections up front
        qT, kT, vT = load_inputs(0)
        tqT, tkT, tv, clos = proj_closures(qT, kT, vT)
        for cl in clos:
            cl()

        for b in range(B_LOC):
            # prefetch the next batch's inputs and defer its projections
            # into this batch's attention stretch (2 tiles per q-tile over
            # heads 1..3 = exactly 48 tiles)
            if b + 1 < B_LOC:
                qTn, kTn, vTn = load_inputs(b + 1)
                tqT_n, tkT_n, tv_n, proj_q = proj_closures(qTn, kTn, vTn)
            else:
                tqT_n = tkT_n = tv_n = None
                proj_q = []

            Z_sb = small.tile([4, 256], f32, tag="Z")
            for h in range(H):
                R_ps = ps_r.tile([4, 1024], f32, tag="R")
                for qt in range(8):
                    sc_ps = ps_sc.tile([128, 1024], f32, tag="sc")
                    for c in range(2):
                        lhs = tqT[:, c, h, qt * 128:(qt + 1) * 128]
                        for nch in range(2):
                            rhs = tkT[:, c, h, nch * 512:(nch + 1) * 512]
                            nc.tensor.matmul(
                                sc_ps[:, nch * 512:(nch + 1) * 512],
                                lhs, rhs,
                                start=(c == 0), stop=(c == 1))
                    # softmax (no max subtraction: |scores| is small)
                    exp_t = expp.tile([128, 1024], bf16, tag="exp")
                    rowsum = stat.tile([128, 1], f32, tag="rs")
                    nc.scalar.activation(exp_t[:], sc_ps[:], AF.Exp,
                                         accum_out=rowsum[:])
                    recip = stat.tile([128, 1], f32, tag="rc")
                    nc.vector.reciprocal(recip[:], rowsum[:])
                    # normalize in bf16 (DVE 4x mode); the f32 conversion for
                    # the attn output happens inside the casting SWDGE store.
                    attn_t = attp.tile([128, 1024], bf16, tag="attn")
                    nc.vector.tensor_scalar_mul(attn_t[:], exp_t[:], recip[:])
                    nc.gpsimd.dma_start(
                        attn_d[b, h, qt * 128:(qt + 1) * 128, :], attn_t[:])
                    # R[j,k] += sum_{q'=j mod 4} attn[q',k]
                    for nch in range(2):
                        nc.tensor.matmul(
                            R_ps[0:4, nch * 512:(nch + 1) * 512],
                            m01_s[:],
                            attn_t[:, nch * 512:(nch + 1) * 512],
                            start=(qt == 0), stop=(qt == 7))
                    if h >= 1:
                        for _ in range(2):
                            if proj_q:
                                proj_q.pop(0)()

                # free R_ps promptly; the RT/Z consumption is deferred
                R_sb = small.tile([4, 1024], f32, tag="Rsb")
                nc.vector.tensor_copy(R_sb[:], R_ps[:])
                if pending:
                    pending.pop(0)()
                pending.append(z_stage(h, R_sb, tv, Z_sb))
            pending.append(out_stage(b, Z_sb))
            while proj_q:
                proj_q.pop(0)()
            tqT, tkT, tv = tqT_n, tkT_n, tv_n

        while pending:
            pending.pop(0)()

    nc.compile()
    return nc


def _get_nc():
    if "nc" not in _CACHE:
        _CACHE["nc"] = _build_nc()
    return _CACHE["nc"]


def _make_in_maps(inputs):
    import ml_dtypes

    bf16 = ml_dtypes.bfloat16
    q = np.asarray(inputs["query"], dtype=np.float32)
    k = np.asarray(inputs["key"], dtype=np.float32)
    v = np.asarray(inputs["value"], dtype=np.float32)
    Wq = np.asarray(inputs["Wq"], dtype=np.float32)
    Wk = np.asarray(inputs["Wk"], dtype=np.float32)
    Wv = np.asarray(inputs["Wv"], dtype=np.float32)
    bq = np.asarray(inputs["bq"], dtype=np.float32)
    bk = np.asarray(inputs["bk"], dtype=np.float32)
    bv = np.asarray(inputs["bv"], dtype=np.float32)
    Wo = np.asarray(inputs["Wo"], dtype=np.float64)
    bo = np.asarray(inputs["bo"], dtype=np.float64)
    W1 = np.asarray(inputs["W1"], dtype=np.float64)
    b1 = np.asarray(inputs["b1"], dtype=np.float64)

    # layout/dtype prep only (all FLOPs of the reference run on device,
    # except the constant folding Wc = Wo @ W1 which is weight-only)
    qT = np.ascontiguousarray(q.transpose(0, 2, 1)).astype(bf16)
    kT = np.ascontiguousarray(
        k.reshape(B, S, 2, 128).transpose(0, 2, 3, 1)).astype(bf16)
    vT = np.ascontiguousarray(
        v.reshape(B, S, 2, 128).transpose(0, 2, 3, 1)).astype(bf16)

    Wc = ((Wo @ W1) / 1024.0).astype(bf16)
    bc = (bo @ W1 + b1).astype(bf16)
    eye = np.eye(128, dtype=np.float32)
    mask01 = np.zeros((128, 4), dtype=bf16)
    mask01[np.arange(128), np.arange(128) % 4] = 1.0

    shared = {"Wq": Wq.astype(bf16), "Wk": Wk.astype(bf16),
              "Wv": Wv.astype(bf16), "Wc": Wc, "bq": bq, "bk": bk,
              "bv": bv.astype(bf16), "bc": bc, "eye": eye, "mask01": mask01}
    in_maps = []
    for c in range(N_CORES):
        sl = slice(c * B_LOC, (c + 1) * B_LOC)
        in_maps.append({"qT": qT[sl], "kT": kT[sl], "vT": vT[sl], **shared})
    return in_maps


def _run(inputs, trace=False, **kw):
    from concourse.bass_utils import run_bass_kernel_spmd

    nc = _get_nc()
    in_maps = _make_in_maps(inputs)
    res = run_bass_kernel_spmd(nc, in_maps, core_ids=list(range(N_CORES)),
                               trace=trace, **kw)
    attn = np.empty((B, H, S, S), dtype=np.float32)
    out = np.empty((B, 128), dtype=np.float32)
    for c in range(N_CORES):
        sl = slice(c * B_LOC, (c + 1) * B_LOC)
        attn[sl] = res.results[c]["attn"]
        out[sl] = res.results[c]["out"]
    return (out, attn), res


def kernel(**inputs):
    (out, attn), _ = _run(inputs)
    return out, attn


# revision 19
# speedup vs baseline: 2.7007x; 1.0646x over previous
"""Trainium2 Bass kernel for nn_MultiHeadCrossAttention_67963562492589.

Reference computation (B=16, S=1024, H=4, QD=128, KD=VD=256):
    tq = (query @ Wq + bq).view(B, H, 1024, 256)   # torch .view semantics!
    tk = (key   @ Wk + bk).view(B, H, 1024, 256)
    tv = (value @ Wv + bv).view(B, H, 1024, 256)
    scores   = tq @ tk^T          (no 1/sqrt(d) scaling)
    attn     = softmax(scores, -1)                  # [B,H,1024,1024] OUTPUT
    attended = attn @ tv  -> .view(B, 1024, 1024)
    out      = ((attended @ Wo + bo).mean(1)) @ W1 + b1   # [B,128] OUTPUT

Key algebraic facts used:
  * The .view head-split means head h covers flat rows h*1024..h*1024+1023 of
    the [4096, 256] projected matrix; flat row r = original position s = r//4
    and feature-quarter j = r%4.  Head h attends over s in [h*256, (h+1)*256).
  * mean-before-matmul: `out` depends on attn only through per-(b,h) column
    sums of attn grouped by (row mod 4):
        R[j, k] = sum_{q = j mod 4} attn[q, k]     (tiny mask matmul, with
                                                    1/rowsum folded into the
                                                    mask weights)
        Z[j, :] = sum_h sum_k R[j,k] * tv_h[k, :]
        out     = (Z.flatten()/1024) @ (Wo @ W1) + (bo @ W1 + b1)
    so `attended` is never materialized and Wo/W1 fold into one [1024,128]
    constant computed on the host (the 1/1024 is folded into it too).

Sharding: pure data parallel - batch 16 -> 2 per core across 8 cores.
Matmuls run in bf16 (PSUM accumulates fp32; fp32r measured 2x slower on
silicon).  The host pre-transposes q/k/v into the [d, s] layouts the
TensorEngine needs and pre-casts inputs/weights to bf16 (pure layout/dtype
prep - every FLOP of the reference computation runs on device).  Softmax is
exp-without-max-subtraction (|scores| stays small for any sane input scale),
with the normalization applied by DVE/GpSimd and folded into the R mask.
"""

import numpy as np

B, S, H = 16, 1024, 4
QD, KD, VD = 128, 256, 256
N_CORES = 8
B_LOC = B // N_CORES  # 2 batches per core

_CACHE = {}


def _build_nc(zero_bias=False):
    import concourse.mybir as mybir
    import concourse.tile as tile
    from concourse import bacc
    from contextlib import ExitStack

    f32 = mybir.dt.float32
    bf16 = mybir.dt.bfloat16
    AF = mybir.ActivationFunctionType

    nc = bacc.Bacc("TRN2", target_bir_lowering=False, debug=False,
                   num_devices=N_CORES)

    # ---- DRAM parameters -------------------------------------------------
    # qT[b]  : [128(qd), 1024(s)]          = query[b].T          (bf16)
    # kT[b]  : [2(c), 128(dl), 1024(s)],  kT[b,c,p,s] = key[b,s,c*128+p]
    # vT[b]  : same layout as kT
    qT_d = nc.dram_tensor("qT", [B_LOC, 128, S], bf16, kind="ExternalInput").ap()
    kT_d = nc.dram_tensor("kT", [B_LOC, 2, 128, S], bf16,
                          kind="ExternalInput").ap()
    vT_d = nc.dram_tensor("vT", [B_LOC, 2, 128, S], bf16,
                          kind="ExternalInput").ap()
    wq_d = nc.dram_tensor("Wq", [QD, H * KD], bf16, kind="ExternalInput").ap()
    wk_d = nc.dram_tensor("Wk", [KD, H * KD], bf16, kind="ExternalInput").ap()
    wv_d = nc.dram_tensor("Wv", [VD, H * VD], bf16, kind="ExternalInput").ap()
    wc_d = nc.dram_tensor("Wc", [1024, 128], bf16, kind="ExternalInput").ap()
    if not zero_bias:
        bq_d = nc.dram_tensor("bq", [1024], f32,
                              kind="ExternalInput").ap()
        bk_d = nc.dram_tensor("bk", [1024], f32,
                              kind="ExternalInput").ap()
        bv_d = nc.dram_tensor("bv", [1024], bf16,
                              kind="ExternalInput").ap()
        bc_d = nc.dram_tensor("bc", [128], bf16,
                              kind="ExternalInput").ap()
    eye_d = nc.dram_tensor("eye", [128, 128], f32, kind="ExternalInput").ap()
    m01_d = nc.dram_tensor("mask01", [128, 4], bf16, kind="ExternalInput").ap()

    attn_d = nc.dram_tensor("attn", [B_LOC, H, S, S], f32,
                            kind="ExternalOutput").ap()
    out_d = nc.dram_tensor("out", [B_LOC, 128], f32,
                           kind="ExternalOutput").ap()

    with tile.TileContext(nc) as tc, ExitStack() as ctx:
        const = ctx.enter_context(tc.tile_pool(name="const", bufs=1))
        trp = ctx.enter_context(tc.tile_pool(name="trp", bufs=2))
        proj = ctx.enter_context(tc.tile_pool(name="proj", bufs=1))
        tvp = ctx.enter_context(tc.tile_pool(name="tvp", bufs=2))
        small = ctx.enter_context(tc.tile_pool(name="small", bufs=2))
        stat = ctx.enter_context(tc.tile_pool(name="stat", bufs=6))
        expp = ctx.enter_context(tc.tile_pool(name="expp", bufs=5))
        attp = ctx.enter_context(tc.tile_pool(name="attp", bufs=5))

        ps_sc = ctx.enter_context(
            tc.tile_pool(name="ps_sc", bufs=2, space="PSUM"))
        ps_r = ctx.enter_context(
            tc.tile_pool(name="ps_r", bufs=1, space="PSUM"))
        ps_m = ctx.enter_context(
            tc.tile_pool(name="ps_m", bufs=2, space="PSUM"))

        # ---- constants / weights into SBUF (all pre-cast on host) --------
        # (emitted after the first batch's input loads below so the first
        # projection matmuls start as early as possible)
        wq_s = const.tile([128, 1024], bf16, tag="wq")
        nc.sync.dma_start(wq_s[:], wq_d[:])
        wk_s = const.tile([128, 2048], bf16, tag="wk")
        nc.sync.dma_start(wk_s[:].rearrange("p (c f) -> p c f", c=2),
                          wk_d.rearrange("(c p) f -> p c f", p=128))
        wv_s = const.tile([128, 2048], bf16, tag="wv")
        nc.sync.dma_start(wv_s[:].rearrange("p (c f) -> p c f", c=2),
                          wv_d.rearrange("(c p) f -> p c f", p=128))
        wc_s = const.tile([128, 1024], bf16, tag="wc")
        nc.sync.dma_start(wc_s[:].rearrange("p (t f) -> p t f", t=8),
                          wc_d.rearrange("(t p) f -> p t f", p=128))
        if not zero_bias:
            bq_c = const.tile([128, 8], f32, tag="bqc")
            nc.sync.dma_start(bq_c[:], bq_d.rearrange("(t p) -> p t", p=128))
            bk_c = const.tile([128, 8], f32, tag="bkc")
            nc.sync.dma_start(bk_c[:], bk_d.rearrange("(t p) -> p t", p=128))
            bv_row = const.tile([1, 1024], bf16, tag="bvr")
            nc.sync.dma_start(bv_row[0:1, :], bv_d[:])
            bc_row = const.tile([1, 128], bf16, tag="bcr")
            nc.sync.dma_start(bc_row[0:1, :], bc_d[:])
        m01_s = const.tile([128, 4], bf16, tag="m01")
        nc.sync.dma_start(m01_s[:], m01_d[:])
        eye_s = const.tile([128, 128], f32, tag="eye")
        nc.sync.dma_start(eye_s[:], eye_d[:])
        ones_s = const.tile([1, 128], bf16, tag="ones")
        nc.vector.memset(ones_s[:], 1.0)

        pending = []  # deferred z/out-stage emission closures

        def load_inputs(b):
            qT = trp.tile([128, 1024], bf16, tag="qT")
            nc.sync.dma_start(qT[:, 0:512], qT_d[b][:, 0:512])
            nc.sync.dma_start(qT[:, 512:1024], qT_d[b][:, 512:1024])
            kT = trp.tile([128, 2, 1024], bf16, tag="kT")
            vT = trp.tile([128, 2, 1024], bf16, tag="vT")
            for c in range(2):
                nc.sync.dma_start(kT[:, c, :], kT_d[b, c])
                nc.sync.dma_start(vT[:, c, :], vT_d[b, c])
            return qT, kT, vT

        evac_ctr = [0]

        def evac(dst, ps, bias_ap=None):
            # PSUM -> SBUF copy (+ optional per-partition bias) + bf16 round,
            # alternating engines to balance DVE/ACT load.
            i = evac_ctr[0]
            evac_ctr[0] += 1
            if bias_ap is None:
                if i % 2 == 0:
                    nc.vector.tensor_copy(dst, ps)
                else:
                    nc.scalar.copy(dst, ps)
            else:
                if i % 2 == 0:
                    nc.vector.tensor_scalar_add(dst, ps, bias_ap)
                else:
                    nc.scalar.activation(dst, ps, AF.Identity, bias=bias_ap)

        def proj_closures(qT, kT, vT):
            """Per-batch projections as a list of single-psum-tile emission
            closures so they can be interleaved into the previous batch's
            attention stretch.  Layouts (q' = qq*4 + j, f = j*256 + c*128 + p,
            s = h*256 + qq):
              tqT[p, c, h, q'] = tq_hT[d = c*128+p, q']   (same for tkT)
              tv[p, st, f]     = tv[s = st*128+p, f]      (natural form)
            """
            tqT = proj.tile([128, 2, 4, 1024], bf16, tag="tqT")
            tkT = proj.tile([128, 2, 4, 1024], bf16, tag="tkT")
            tv = tvp.tile([128, 8, 1024], bf16, tag="tv")
            tqT_w = tqT[:].rearrange("p c hh (qq j) -> p c hh qq j", j=4)
            tkT_w = tkT[:].rearrange("p c hh (qq j) -> p c hh qq j", j=4)
            clos = []

            def q_tile(m, sc):
                j, c = m // 2, m % 2
                ps = ps_m.tile([128, 512], f32, tag="misc")
                nc.tensor.matmul(
                    ps[:], wq_s[:, m * 128:(m + 1) * 128],
                    qT[:, sc * 512:(sc + 1) * 512],
                    start=True, stop=True)
                evac(tqT_w[:, c, sc * 2:(sc + 1) * 2, :, j], ps[:],
                     None if zero_bias else bq_c[:, m:m + 1])

            def k_tile(m, sc):
                j, c2 = m // 2, m % 2
                ps = ps_m.tile([128, 512], f32, tag="misc")
                for c in range(2):
                    nc.tensor.matmul(
                        ps[:],
                        wk_s[:, c * 1024 + m * 128:c * 1024 + (m + 1) * 128],
                        kT[:, c, sc * 512:(sc + 1) * 512],
                        start=(c == 0), stop=(c == 1))
                evac(tkT_w[:, c2, sc * 2:(sc + 1) * 2, :, j], ps[:],
                     None if zero_bias else bk_c[:, m:m + 1])

            def v_tile(st, fc):
                ps = ps_m.tile([128, 512], f32, tag="misc")
                for c in range(2):
                    nc.tensor.matmul(
                        ps[:],
                        vT[:, c, st * 128:(st + 1) * 128],
                        wv_s[:, c * 1024 + fc * 512:c * 1024 + fc * 512 + 512],
                        start=(c == 0), stop=(zero_bias and c == 1))
                if not zero_bias:
                    # + bv broadcast along partitions via rank-1 accumulate
                    nc.tensor.matmul(
                        ps[:], ones_s[0:1, :],
                        bv_row[0:1, fc * 512:(fc + 1) * 512],
                        start=False, stop=True)
                evac(tv[:, st, fc * 512:(fc + 1) * 512], ps[:])

            for m in range(8):
                for sc in range(2):
                    clos.append(lambda m=m, sc=sc: q_tile(m, sc))
            for m in range(8):
                for sc in range(2):
                    clos.append(lambda m=m, sc=sc: k_tile(m, sc))
            for st in range(8):
                for fc in range(2):
                    clos.append(lambda st=st, fc=fc: v_tile(st, fc))
            return tqT, tkT, tv, clos

        def z_stage(h, R_sb, tv, Z_sb):
            # The R->RT->Z chain is a latency-bound PE<->DVE ping-pong, so it
            # is emitted one head LATE (into the next head's scores stretch):
            # PE reaches it with every dependency long satisfied.
            def emit():
                R_r = R_sb[:].rearrange("p (kk i) -> p i kk", i=4)
                RT = small.tile([128, 8, 4], bf16, tag="RT")
                for i in range(4):
                    for kc in range(2):
                        ps = ps_m.tile([128, 128], f32, tag="misc")
                        nc.tensor.transpose(
                            ps[0:128, 0:4],
                            R_r[0:4, i, kc * 128:(kc + 1) * 128],
                            eye_s[0:4, 0:4])
                        nc.vector.tensor_copy(RT[:, i * 2 + kc, :],
                                              ps[0:128, 0:4])
                Z_ps = ps_m.tile([128, 512], f32, tag="misc")
                n = 0
                for i in range(4):
                    for kc in range(2):
                        nc.tensor.matmul(
                            Z_ps[0:4, 0:256],
                            RT[:, i * 2 + kc, :],
                            tv[:, 2 * h + kc, i * 256:(i + 1) * 256],
                            start=(n == 0), stop=(n == 7))
                        n += 1
                if h == 0:
                    nc.vector.tensor_copy(Z_sb[:], Z_ps[0:4, 0:256])
                else:
                    nc.vector.tensor_add(Z_sb[:], Z_sb[:], Z_ps[0:4, 0:256])
            return emit

        def out_stage(b, Z_sb):
            def emit():
                # out[b] = (Z.flat/1024) @ (Wo@W1) + bc  (1/1024 in Wc);
                # attmean[256*j + hf*128 + p] = Z[j, hf*128+p] = zT[hf][p, j]
                zT = small.tile([128, 2, 4], bf16, tag="zT")
                for hf in range(2):
                    ps = ps_m.tile([128, 128], f32, tag="misc")
                    nc.tensor.transpose(
                        ps[0:128, 0:4],
                        Z_sb[0:4, hf * 128:(hf + 1) * 128],
                        eye_s[0:4, 0:4])
                    nc.vector.tensor_copy(zT[:, hf, :], ps[0:128, 0:4])
                out_ps = ps_m.tile([128, 512], f32, tag="misc")
                for t in range(8):
                    j, hf = t // 2, t % 2
                    nc.tensor.matmul(
                        out_ps[0:1, 0:128],
                        zT[:, hf, j:j + 1],
                        wc_s[:, t * 128:(t + 1) * 128],
                        start=(t == 0), stop=(zero_bias and t == 7))
                if not zero_bias:
                    nc.tensor.matmul(out_ps[0:1, 0:128], ones_s[0:1, 0:1],
                                     bc_row[0:1, :], start=False, stop=True)
                out_sb = small.tile([1, 128], f32, tag="outsb")
                nc.vector.tensor_copy(out_sb[:], out_ps[0:1, 0:128])
                nc.sync.dma_start(out_d[b:b + 1, :], out_sb[:])
            return emit

        # batch 0: inputs + projections up front
        qT, kT, vT = load_inputs(0)
        tqT, tkT, tv, clos = proj_closures(qT, kT, vT)
        for cl in clos:
            cl()

        for b in range(B_LOC):
            # prefetch the next batch's inputs and defer its projections
            # into this batch's attention stretch (2 tiles per q-tile over
            # heads 1..3 = exactly 48 tiles)
            if b + 1 < B_LOC:
                qTn, kTn, vTn = load_inputs(b + 1)
                tqT_n, tkT_n, tv_n, proj_q = proj_closures(qTn, kTn, vTn)
            else:
                tqT_n = tkT_n = tv_n = None
                proj_q = []

            Z_sb = small.tile([4, 256], f32, tag="Z")
            for h in range(H):
                R_ps = ps_r.tile([4, 1024], f32, tag="R")
                for qt in range(8):
                    sc_ps = ps_sc.tile([128, 1024], f32, tag="sc")
                    for c in range(2):
                        lhs = tqT[:, c, h, qt * 128:(qt + 1) * 128]
                        for nch in range(2):
                            rhs = tkT[:, c, h, nch * 512:(nch + 1) * 512]
                            nc.tensor.matmul(
                                sc_ps[:, nch * 512:(nch + 1) * 512],
                                lhs, rhs,
                                start=(c == 0), stop=(c == 1))
                    # softmax (no max subtraction: |scores| is small)
                    exp_t = expp.tile([128, 1024], bf16, tag="exp")
                    rowsum = stat.tile([128, 1], f32, tag="rs")
                    nc.scalar.activation(exp_t[:], sc_ps[:], AF.Exp,
                                         accum_out=rowsum[:])
                    recip = stat.tile([128, 1], f32, tag="rc")
                    nc.vector.reciprocal(recip[:], rowsum[:])
                    # normalize in bf16 (DVE 4x mode); the f32 conversion for
                    # the attn output happens inside the casting SWDGE store.
                    attn_t = attp.tile([128, 1024], bf16, tag="attn")
                    nc.vector.tensor_scalar_mul(attn_t[:], exp_t[:], recip[:])
                    nc.gpsimd.dma_start(
                        attn_d[b, h, qt * 128:(qt + 1) * 128, :], attn_t[:])
                    # R[j,k] += sum_{q'=j mod 4} attn[q',k]
                    for nch in range(2):
                        nc.tensor.matmul(
                            R_ps[0:4, nch * 512:(nch + 1) * 512],
                            m01_s[:],
                            attn_t[:, nch * 512:(nch + 1) * 512],
                            start=(qt == 0), stop=(qt == 7))
                    if h >= 1:
                        for _ in range(2):
                            if proj_q:
                                proj_q.pop(0)()
                    if b == B_LOC - 1 and qt == 3 and pending:
                        pending.pop(0)()

                # free R_ps promptly; the RT/Z consumption is deferred
                R_sb = small.tile([4, 1024], f32, tag="Rsb")
                nc.vector.tensor_copy(R_sb[:], R_ps[:])
                if pending:
                    pending.pop(0)()
                pending.append(z_stage(h, R_sb, tv, Z_sb))
            pending.append(out_stage(b, Z_sb))
            while proj_q:
                proj_q.pop(0)()
            tqT, tkT, tv = tqT_n, tkT_n, tv_n

        while pending:
            pending.pop(0)()

    nc.compile()
    return nc


def _get_nc(zero_bias):
    key = ("nc", zero_bias)
    if key not in _CACHE:
        _CACHE[key] = _build_nc(zero_bias=zero_bias)
    return _CACHE[key]


def _make_in_maps(inputs):
    import ml_dtypes

    bf16 = ml_dtypes.bfloat16
    q = np.asarray(inputs["query"], dtype=np.float32)
    k = np.asarray(inputs["key"], dtype=np.float32)
    v = np.asarray(inputs["value"], dtype=np.float32)
    Wq = np.asarray(inputs["Wq"], dtype=np.float32)
    Wk = np.asarray(inputs["Wk"], dtype=np.float32)
    Wv = np.asarray(inputs["Wv"], dtype=np.float32)
    bq = np.asarray(inputs["bq"], dtype=np.float32)
    bk = np.asarray(inputs["bk"], dtype=np.float32)
    bv = np.asarray(inputs["bv"], dtype=np.float32)
    Wo = np.asarray(inputs["Wo"], dtype=np.float64)
    bo = np.asarray(inputs["bo"], dtype=np.float64)
    W1 = np.asarray(inputs["W1"], dtype=np.float64)
    b1 = np.asarray(inputs["b1"], dtype=np.float64)

    # layout/dtype prep only (all FLOPs of the reference run on device,
    # except the constant folding Wc = Wo @ W1 which is weight-only)
    qT = np.ascontiguousarray(q.transpose(0, 2, 1)).astype(bf16)
    kT = np.ascontiguousarray(
        k.reshape(B, S, 2, 128).transpose(0, 2, 3, 1)).astype(bf16)
    vT = np.ascontiguousarray(
        v.reshape(B, S, 2, 128).transpose(0, 2, 3, 1)).astype(bf16)

    Wc = ((Wo @ W1) / 1024.0).astype(bf16)
    bc = (bo @ W1 + b1).astype(bf16)
    eye = np.eye(128, dtype=np.float32)
    mask01 = np.zeros((128, 4), dtype=bf16)
    mask01[np.arange(128), np.arange(128) % 4] = 1.0

    zero_bias = not (bq.any() or bk.any() or bv.any() or bo.any()
                     or b1.any())
    shared = {"Wq": Wq.astype(bf16), "Wk": Wk.astype(bf16),
              "Wv": Wv.astype(bf16), "Wc": Wc, "eye": eye, "mask01": mask01}
    if not zero_bias:
        shared.update({"bq": bq, "bk": bk, "bv": bv.astype(bf16), "bc": bc})
    in_maps = []
    for c in range(N_CORES):
        sl = slice(c * B_LOC, (c + 1) * B_LOC)
        in_maps.append({"qT": qT[sl], "kT": kT[sl], "vT": vT[sl], **shared})
    return in_maps, zero_bias


def _run(inputs, trace=False, **kw):
    from concourse.bass_utils import run_bass_kernel_spmd

    in_maps, zero_bias = _make_in_maps(inputs)
    nc = _get_nc(zero_bias)
    res = run_bass_kernel_spmd(nc, in_maps, core_ids=list(range(N_CORES)),
                               trace=trace, **kw)
    attn = np.empty((B, H, S, S), dtype=np.float32)
    out = np.empty((B, 128), dtype=np.float32)
    for c in range(N_CORES):
        sl = slice(c * B_LOC, (c + 1) * B_LOC)
        attn[sl] = res.results[c]["attn"]
        out[sl] = res.results[c]["out"]
    return (out, attn), res


def kernel(**inputs):
    (out, attn), _ = _run(inputs)
    return out, attn


# revision 20
# speedup vs baseline: 2.7586x; 1.0214x over previous
"""Trainium2 Bass kernel for nn_MultiHeadCrossAttention_67963562492589.

Reference computation (B=16, S=1024, H=4, QD=128, KD=VD=256):
    tq = (query @ Wq + bq).view(B, H, 1024, 256)   # torch .view semantics!
    tk = (key   @ Wk + bk).view(B, H, 1024, 256)
    tv = (value @ Wv + bv).view(B, H, 1024, 256)
    scores   = tq @ tk^T          (no 1/sqrt(d) scaling)
    attn     = softmax(scores, -1)                  # [B,H,1024,1024] OUTPUT
    attended = attn @ tv  -> .view(B, 1024, 1024)
    out      = ((attended @ Wo + bo).mean(1)) @ W1 + b1   # [B,128] OUTPUT

Key algebraic facts used:
  * The .view head-split means head h covers flat rows h*1024..h*1024+1023 of
    the [4096, 256] projected matrix; flat row r = original position s = r//4
    and feature-quarter j = r%4.  Head h attends over s in [h*256, (h+1)*256).
  * mean-before-matmul: `out` depends on attn only through per-(b,h) column
    sums of attn grouped by (row mod 4):
        R[j, k] = sum_{q = j mod 4} attn[q, k]     (tiny mask matmul, with
                                                    1/rowsum folded into the
                                                    mask weights)
        Z[j, :] = sum_h sum_k R[j,k] * tv_h[k, :]
        out     = (Z.flatten()/1024) @ (Wo @ W1) + (bo @ W1 + b1)
    so `attended` is never materialized and Wo/W1 fold into one [1024,128]
    constant computed on the host (the 1/1024 is folded into it too).

Sharding: pure data parallel - batch 16 -> 2 per core across 8 cores.
Matmuls run in bf16 (PSUM accumulates fp32; fp32r measured 2x slower on
silicon).  The host pre-transposes q/k/v into the [d, s] layouts the
TensorEngine needs and pre-casts inputs/weights to bf16 (pure layout/dtype
prep - every FLOP of the reference computation runs on device).  Softmax is
exp-without-max-subtraction (|scores| stays small for any sane input scale),
with the normalization applied by DVE/GpSimd and folded into the R mask.
"""

import numpy as np

B, S, H = 16, 1024, 4
QD, KD, VD = 128, 256, 256
N_CORES = 8
B_LOC = B // N_CORES  # 2 batches per core

_CACHE = {}


def _build_nc(zero_bias=False):
    import concourse.mybir as mybir
    import concourse.tile as tile
    from concourse import bacc
    from contextlib import ExitStack

    f32 = mybir.dt.float32
    bf16 = mybir.dt.bfloat16
    AF = mybir.ActivationFunctionType

    nc = bacc.Bacc("TRN2", target_bir_lowering=False, debug=False,
                   num_devices=N_CORES)

    # ---- DRAM parameters -------------------------------------------------
    # qT[b]  : [128(qd), 1024(s)]          = query[b].T          (bf16)
    # kT[b]  : [2(c), 128(dl), 1024(s)],  kT[b,c,p,s] = key[b,s,c*128+p]
    # vT[b]  : same layout as kT
    qT_d = nc.dram_tensor("qT", [B_LOC, 128, S], bf16, kind="ExternalInput").ap()
    kT_d = nc.dram_tensor("kT", [B_LOC, 2, 128, S], bf16,
                          kind="ExternalInput").ap()
    vT_d = nc.dram_tensor("vT", [B_LOC, 2, 128, S], bf16,
                          kind="ExternalInput").ap()
    wq_d = nc.dram_tensor("Wq", [QD, H * KD], bf16, kind="ExternalInput").ap()
    wk_d = nc.dram_tensor("Wk", [KD, H * KD], bf16, kind="ExternalInput").ap()
    wv_d = nc.dram_tensor("Wv", [VD, H * VD], bf16, kind="ExternalInput").ap()
    wc_d = nc.dram_tensor("Wc", [1024, 128], bf16, kind="ExternalInput").ap()
    if not zero_bias:
        bq_d = nc.dram_tensor("bq", [1024], f32,
                              kind="ExternalInput").ap()
        bk_d = nc.dram_tensor("bk", [1024], f32,
                              kind="ExternalInput").ap()
        bv_d = nc.dram_tensor("bv", [1024], bf16,
                              kind="ExternalInput").ap()
        bc_d = nc.dram_tensor("bc", [128], bf16,
                              kind="ExternalInput").ap()
    eye_d = nc.dram_tensor("eye", [128, 128], f32, kind="ExternalInput").ap()
    m01_d = nc.dram_tensor("mask01", [128, 4], bf16, kind="ExternalInput").ap()

    attn_d = nc.dram_tensor("attn", [B_LOC, H, S, S], f32,
                            kind="ExternalOutput").ap()
    out_d = nc.dram_tensor("out", [B_LOC, 128], f32,
                           kind="ExternalOutput").ap()

    with tile.TileContext(nc) as tc, ExitStack() as ctx:
        const = ctx.enter_context(tc.tile_pool(name="const", bufs=1))
        trp = ctx.enter_context(tc.tile_pool(name="trp", bufs=2))
        proj = ctx.enter_context(tc.tile_pool(name="proj", bufs=1))
        tvp = ctx.enter_context(tc.tile_pool(name="tvp", bufs=2))
        small = ctx.enter_context(tc.tile_pool(name="small", bufs=2))
        stat = ctx.enter_context(tc.tile_pool(name="stat", bufs=6))
        expp = ctx.enter_context(tc.tile_pool(name="expp", bufs=5))
        attp = ctx.enter_context(tc.tile_pool(name="attp", bufs=5))

        ps_sc = ctx.enter_context(
            tc.tile_pool(name="ps_sc", bufs=2, space="PSUM"))
        ps_r = ctx.enter_context(
            tc.tile_pool(name="ps_r", bufs=1, space="PSUM"))
        ps_m = ctx.enter_context(
            tc.tile_pool(name="ps_m", bufs=2, space="PSUM"))

        # ---- constants / weights into SBUF (all pre-cast on host) --------
        # DMA issue order is chosen so the first projection matmuls (wq x qT)
        # have their inputs as early as possible.
        wq_s = const.tile([128, 1024], bf16, tag="wq")
        nc.sync.dma_start(wq_s[:], wq_d[:])

        def load_inputs(b):
            qT = trp.tile([128, 1024], bf16, tag="qT")
            nc.sync.dma_start(qT[:, 0:512], qT_d[b][:, 0:512])
            nc.sync.dma_start(qT[:, 512:1024], qT_d[b][:, 512:1024])
            kT = trp.tile([128, 2, 1024], bf16, tag="kT")
            vT = trp.tile([128, 2, 1024], bf16, tag="vT")
            for c in range(2):
                nc.sync.dma_start(kT[:, c, :], kT_d[b, c])
                nc.sync.dma_start(vT[:, c, :], vT_d[b, c])
            return qT, kT, vT

        in0 = load_inputs(0)

        wk_s = const.tile([128, 2048], bf16, tag="wk")
        nc.sync.dma_start(wk_s[:].rearrange("p (c f) -> p c f", c=2),
                          wk_d.rearrange("(c p) f -> p c f", p=128))
        wv_s = const.tile([128, 2048], bf16, tag="wv")
        nc.sync.dma_start(wv_s[:].rearrange("p (c f) -> p c f", c=2),
                          wv_d.rearrange("(c p) f -> p c f", p=128))
        wc_s = const.tile([128, 1024], bf16, tag="wc")
        nc.sync.dma_start(wc_s[:].rearrange("p (t f) -> p t f", t=8),
                          wc_d.rearrange("(t p) f -> p t f", p=128))
        if not zero_bias:
            bq_c = const.tile([128, 8], f32, tag="bqc")
            nc.sync.dma_start(bq_c[:], bq_d.rearrange("(t p) -> p t", p=128))
            bk_c = const.tile([128, 8], f32, tag="bkc")
            nc.sync.dma_start(bk_c[:], bk_d.rearrange("(t p) -> p t", p=128))
            bv_row = const.tile([1, 1024], bf16, tag="bvr")
            nc.sync.dma_start(bv_row[0:1, :], bv_d[:])
            bc_row = const.tile([1, 128], bf16, tag="bcr")
            nc.sync.dma_start(bc_row[0:1, :], bc_d[:])
        m01_s = const.tile([128, 4], bf16, tag="m01")
        nc.sync.dma_start(m01_s[:], m01_d[:])
        eye_s = const.tile([128, 128], f32, tag="eye")
        nc.sync.dma_start(eye_s[:], eye_d[:])
        ones_s = const.tile([1, 128], bf16, tag="ones")
        nc.vector.memset(ones_s[:], 1.0)

        pending = []  # deferred z/out-stage emission closures

        evac_ctr = [0]

        def evac(dst, ps, bias_ap=None):
            # PSUM -> SBUF copy (+ optional per-partition bias) + bf16 round,
            # alternating engines to balance DVE/ACT load.
            i = evac_ctr[0]
            evac_ctr[0] += 1
            if bias_ap is None:
                if i % 2 == 0:
                    nc.vector.tensor_copy(dst, ps)
                else:
                    nc.scalar.copy(dst, ps)
            else:
                if i % 2 == 0:
                    nc.vector.tensor_scalar_add(dst, ps, bias_ap)
                else:
                    nc.scalar.activation(dst, ps, AF.Identity, bias=bias_ap)

        def proj_closures(qT, kT, vT):
            """Per-batch projections as a list of single-psum-tile emission
            closures so they can be interleaved into the previous batch's
            attention stretch.  Layouts (q' = qq*4 + j, f = j*256 + c*128 + p,
            s = h*256 + qq):
              tqT[p, c, h, q'] = tq_hT[d = c*128+p, q']   (same for tkT)
              tv[p, st, f]     = tv[s = st*128+p, f]      (natural form)
            """
            tqT = proj.tile([128, 2, 4, 1024], bf16, tag="tqT")
            tkT = proj.tile([128, 2, 4, 1024], bf16, tag="tkT")
            tv = tvp.tile([128, 8, 1024], bf16, tag="tv")
            tqT_w = tqT[:].rearrange("p c hh (qq j) -> p c hh qq j", j=4)
            tkT_w = tkT[:].rearrange("p c hh (qq j) -> p c hh qq j", j=4)
            clos = []

            def q_tile(m, sc):
                j, c = m // 2, m % 2
                ps = ps_m.tile([128, 512], f32, tag="misc")
                nc.tensor.matmul(
                    ps[:], wq_s[:, m * 128:(m + 1) * 128],
                    qT[:, sc * 512:(sc + 1) * 512],
                    start=True, stop=True)
                evac(tqT_w[:, c, sc * 2:(sc + 1) * 2, :, j], ps[:],
                     None if zero_bias else bq_c[:, m:m + 1])

            def k_tile(m, sc):
                j, c2 = m // 2, m % 2
                ps = ps_m.tile([128, 512], f32, tag="misc")
                for c in range(2):
                    nc.tensor.matmul(
                        ps[:],
                        wk_s[:, c * 1024 + m * 128:c * 1024 + (m + 1) * 128],
                        kT[:, c, sc * 512:(sc + 1) * 512],
                        start=(c == 0), stop=(c == 1))
                evac(tkT_w[:, c2, sc * 2:(sc + 1) * 2, :, j], ps[:],
                     None if zero_bias else bk_c[:, m:m + 1])

            def v_tile(st, fc):
                ps = ps_m.tile([128, 512], f32, tag="misc")
                for c in range(2):
                    nc.tensor.matmul(
                        ps[:],
                        vT[:, c, st * 128:(st + 1) * 128],
                        wv_s[:, c * 1024 + fc * 512:c * 1024 + fc * 512 + 512],
                        start=(c == 0), stop=(zero_bias and c == 1))
                if not zero_bias:
                    # + bv broadcast along partitions via rank-1 accumulate
                    nc.tensor.matmul(
                        ps[:], ones_s[0:1, :],
                        bv_row[0:1, fc * 512:(fc + 1) * 512],
                        start=False, stop=True)
                evac(tv[:, st, fc * 512:(fc + 1) * 512], ps[:])

            for m in range(8):
                for sc in range(2):
                    clos.append(lambda m=m, sc=sc: q_tile(m, sc))
            for m in range(8):
                for sc in range(2):
                    clos.append(lambda m=m, sc=sc: k_tile(m, sc))
            for st in range(8):
                for fc in range(2):
                    clos.append(lambda st=st, fc=fc: v_tile(st, fc))
            return tqT, tkT, tv, clos

        def z_stage(h, R_sb, tv, Z_sb):
            # The R->RT->Z chain is a latency-bound PE<->DVE ping-pong, so it
            # is emitted one head LATE (into the next head's scores stretch):
            # PE reaches it with every dependency long satisfied.
            def emit():
                R_r = R_sb[:].rearrange("p (kk i) -> p i kk", i=4)
                RT = small.tile([128, 8, 4], bf16, tag="RT")
                for i in range(4):
                    for kc in range(2):
                        ps = ps_m.tile([128, 128], f32, tag="misc")
                        nc.tensor.transpose(
                            ps[0:128, 0:4],
                            R_r[0:4, i, kc * 128:(kc + 1) * 128],
                            eye_s[0:4, 0:4])
                        nc.vector.tensor_copy(RT[:, i * 2 + kc, :],
                                              ps[0:128, 0:4])
                Z_ps = ps_m.tile([128, 512], f32, tag="misc")
                n = 0
                for i in range(4):
                    for kc in range(2):
                        nc.tensor.matmul(
                            Z_ps[0:4, 0:256],
                            RT[:, i * 2 + kc, :],
                            tv[:, 2 * h + kc, i * 256:(i + 1) * 256],
                            start=(n == 0), stop=(n == 7))
                        n += 1
                if h == 0:
                    nc.vector.tensor_copy(Z_sb[:], Z_ps[0:4, 0:256])
                else:
                    nc.vector.tensor_add(Z_sb[:], Z_sb[:], Z_ps[0:4, 0:256])
            return emit

        def out_stage(b, Z_sb):
            def emit():
                # out[b] = (Z.flat/1024) @ (Wo@W1) + bc  (1/1024 in Wc);
                # attmean[256*j + hf*128 + p] = Z[j, hf*128+p] = zT[hf][p, j]
                zT = small.tile([128, 2, 4], bf16, tag="zT")
                for hf in range(2):
                    ps = ps_m.tile([128, 128], f32, tag="misc")
                    nc.tensor.transpose(
                        ps[0:128, 0:4],
                        Z_sb[0:4, hf * 128:(hf + 1) * 128],
                        eye_s[0:4, 0:4])
                    nc.vector.tensor_copy(zT[:, hf, :], ps[0:128, 0:4])
                out_ps = ps_m.tile([128, 512], f32, tag="misc")
                for t in range(8):
                    j, hf = t // 2, t % 2
                    nc.tensor.matmul(
                        out_ps[0:1, 0:128],
                        zT[:, hf, j:j + 1],
                        wc_s[:, t * 128:(t + 1) * 128],
                        start=(t == 0), stop=(zero_bias and t == 7))
                if not zero_bias:
                    nc.tensor.matmul(out_ps[0:1, 0:128], ones_s[0:1, 0:1],
                                     bc_row[0:1, :], start=False, stop=True)
                out_sb = small.tile([1, 128], f32, tag="outsb")
                nc.vector.tensor_copy(out_sb[:], out_ps[0:1, 0:128])
                nc.sync.dma_start(out_d[b:b + 1, :], out_sb[:])
            return emit

        # batch 0: inputs (loaded above) + projections up front
        qT, kT, vT = in0
        tqT, tkT, tv, clos = proj_closures(qT, kT, vT)
        for cl in clos:
            cl()

        for b in range(B_LOC):
            # prefetch the next batch's inputs and defer its projections
            # into this batch's attention stretch (2 tiles per q-tile over
            # heads 1..3 = exactly 48 tiles)
            if b + 1 < B_LOC:
                qTn, kTn, vTn = load_inputs(b + 1)
                tqT_n, tkT_n, tv_n, proj_q = proj_closures(qTn, kTn, vTn)
            else:
                tqT_n = tkT_n = tv_n = None
                proj_q = []

            Z_sb = small.tile([4, 256], f32, tag="Z")
            for h in range(H):
                R_ps = ps_r.tile([4, 1024], f32, tag="R")
                for qt in range(8):
                    sc_ps = ps_sc.tile([128, 1024], f32, tag="sc")
                    for c in range(2):
                        lhs = tqT[:, c, h, qt * 128:(qt + 1) * 128]
                        for nch in range(2):
                            rhs = tkT[:, c, h, nch * 512:(nch + 1) * 512]
                            nc.tensor.matmul(
                                sc_ps[:, nch * 512:(nch + 1) * 512],
                                lhs, rhs,
                                start=(c == 0), stop=(c == 1))
                    # softmax (no max subtraction: |scores| is small)
                    exp_t = expp.tile([128, 1024], bf16, tag="exp")
                    rowsum = stat.tile([128, 1], f32, tag="rs")
                    nc.scalar.activation(exp_t[:], sc_ps[:], AF.Exp,
                                         accum_out=rowsum[:])
                    recip = stat.tile([128, 1], f32, tag="rc")
                    nc.vector.reciprocal(recip[:], rowsum[:])
                    # normalize in bf16 (DVE 4x mode); the f32 conversion for
                    # the attn output happens inside the casting SWDGE store.
                    attn_t = attp.tile([128, 1024], bf16, tag="attn")
                    nc.vector.tensor_scalar_mul(attn_t[:], exp_t[:], recip[:])
                    nc.gpsimd.dma_start(
                        attn_d[b, h, qt * 128:(qt + 1) * 128, :], attn_t[:])
                    # R[j,k] += sum_{q'=j mod 4} attn[q',k]
                    for nch in range(2):
                        nc.tensor.matmul(
                            R_ps[0:4, nch * 512:(nch + 1) * 512],
                            m01_s[:],
                            attn_t[:, nch * 512:(nch + 1) * 512],
                            start=(qt == 0), stop=(qt == 7))
                    if h >= 1:
                        for _ in range(2):
                            if proj_q:
                                proj_q.pop(0)()
                    if b == B_LOC - 1 and qt == 3 and pending:
                        pending.pop(0)()

                # free R_ps promptly; the RT/Z consumption is deferred
                R_sb = small.tile([4, 1024], f32, tag="Rsb")
                nc.vector.tensor_copy(R_sb[:], R_ps[:])
                if pending:
                    pending.pop(0)()
                pending.append(z_stage(h, R_sb, tv, Z_sb))
            pending.append(out_stage(b, Z_sb))
            while proj_q:
                proj_q.pop(0)()
            tqT, tkT, tv = tqT_n, tkT_n, tv_n

        while pending:
            pending.pop(0)()

    nc.compile()
    return nc


def _get_nc(zero_bias):
    key = ("nc", zero_bias)
    if key not in _CACHE:
        _CACHE[key] = _build_nc(zero_bias=zero_bias)
    return _CACHE[key]


def _make_in_maps(inputs):
    import ml_dtypes

    bf16 = ml_dtypes.bfloat16
    q = np.asarray(inputs["query"], dtype=np.float32)
    k = np.asarray(inputs["key"], dtype=np.float32)
    v = np.asarray(inputs["value"], dtype=np.float32)
    Wq = np.asarray(inputs["Wq"], dtype=np.float32)
    Wk = np.asarray(inputs["Wk"], dtype=np.float32)
    Wv = np.asarray(inputs["Wv"], dtype=np.float32)
    bq = np.asarray(inputs["bq"], dtype=np.float32)
    bk = np.asarray(inputs["bk"], dtype=np.float32)
    bv = np.asarray(inputs["bv"], dtype=np.float32)
    Wo = np.asarray(inputs["Wo"], dtype=np.float64)
    bo = np.asarray(inputs["bo"], dtype=np.float64)
    W1 = np.asarray(inputs["W1"], dtype=np.float64)
    b1 = np.asarray(inputs["b1"], dtype=np.float64)

    # layout/dtype prep only (all FLOPs of the reference run on device,
    # except the constant folding Wc = Wo @ W1 which is weight-only)
    qT = np.ascontiguousarray(q.transpose(0, 2, 1)).astype(bf16)
    kT = np.ascontiguousarray(
        k.reshape(B, S, 2, 128).transpose(0, 2, 3, 1)).astype(bf16)
    vT = np.ascontiguousarray(
        v.reshape(B, S, 2, 128).transpose(0, 2, 3, 1)).astype(bf16)

    Wc = ((Wo @ W1) / 1024.0).astype(bf16)
    bc = (bo @ W1 + b1).astype(bf16)
    eye = np.eye(128, dtype=np.float32)
    mask01 = np.zeros((128, 4), dtype=bf16)
    mask01[np.arange(128), np.arange(128) % 4] = 1.0

    zero_bias = not (bq.any() or bk.any() or bv.any() or bo.any()
                     or b1.any())
    shared = {"Wq": Wq.astype(bf16), "Wk": Wk.astype(bf16),
              "Wv": Wv.astype(bf16), "Wc": Wc, "eye": eye, "mask01": mask01}
    if not zero_bias:
        shared.update({"bq": bq, "bk": bk, "bv": bv.astype(bf16), "bc": bc})
    in_maps = []
    for c in range(N_CORES):
        sl = slice(c * B_LOC, (c + 1) * B_LOC)
        in_maps.append({"qT": qT[sl], "kT": kT[sl], "vT": vT[sl], **shared})
    return in_maps, zero_bias


def _run(inputs, trace=False, **kw):
    from concourse.bass_utils import run_bass_kernel_spmd

    in_maps, zero_bias = _make_in_maps(inputs)
    nc = _get_nc(zero_bias)
    res = run_bass_kernel_spmd(nc, in_maps, core_ids=list(range(N_CORES)),
                               trace=trace, **kw)
    attn = np.empty((B, H, S, S), dtype=np.float32)
    out = np.empty((B, 128), dtype=np.float32)
    for c in range(N_CORES):
        sl = slice(c * B_LOC, (c + 1) * B_LOC)
        attn[sl] = res.results[c]["attn"]
        out[sl] = res.results[c]["out"]
    return (out, attn), res


def kernel(**inputs):
    (out, attn), _ = _run(inputs)
    return out, attn
